# revision 20
# baseline (speedup 1.0000x reference)
"""Cross-attention kernel for 8 Trainium2 NeuronCores (Bass/Tile, SPMD).

Problem: nn_CrossAttention (B=4, NQ=1024, NK=2048, C=1024, H=16, D=64), fp32.

Sharding: (batch x head-group) across the 8 cores. Core c handles batch
b = c//2 and heads h0 = (c%2)*8 .. h0+8 (column-parallel q/k/v projections,
row-parallel output projection). Each core emits a partial output
projection [NQ, C]; the host sums the two partials per batch (+ biases).

Device dataflow is fully "feature-major" (transposed): the host passes
x.T / y.T / W.T so every matmul contraction runs over the SBUF partition
axis with no on-device transposes:

  qT[o,t]  = sum_c wqT[c,o] * xT[c,t]          (o-blocks of 128 = 2 heads)
  kT[o,s]  = sum_c wkT[c,o] * yT[c,s]
  v[s,o]   = sum_c yT[c,s] * wvT[c,o]          (token-major)
  ST[s,t]  = sum_d kT_h[d,s-chunk] * qT_h[d,t]   per head (K=64)
  P[s,t]   = exp(ST) * mask01[s,t]               (ACT exp from PSUM, DVE mul)
  out_aug  = sum_s [ones | v_h]^T P  -> rows 0:64 denominator (x64
             replicated), rows 64:128 numerator
  outF     = out_aug[64:128] * recip(out_aug[0:64])   (custom-DVE recip
             reads PSUM directly; no partition broadcast / row extract)
  partial[t,co] = sum_o outF[o,t-block] * wpT[o,co]

All projections are bf16 (fp8 was tried: softmax score noise transfers
~1:1 into the output - no sqrt(N) washout - and e4m3's ~3% per-operand
quantization lands the end-to-end error at 3.2e-2 > the 2e-2 gate).

Schedule: fully software-pipelined. The attention loop processes the 8
heads sequentially; v-projection and the kT/qT projections for later head
pairs run as PE "filler" inside the chunk loop so the tensor engine stays
busy while the ACT engine drains the exp stream. LA=3 chunk lookahead
covers the exp+mask latency and the early-DMA arrival of v. Iterations
with no filler emit one dummy matmul - without it the PE micro-idles every
chunk in the ACT-bound passes and the HAM clock-gate drops the PE to 1.2
GHz (measured: ~70us of half-clock). Lead-in DMAs are spread across five
engine queues so the first-score critical path (~5MB of y/wk/x/wq/m0) does
not serialize on one queue. PSUM: 2 banks projections + 4 banks scores
(2 in flight) + 2 banks attn-out = 8.

Bias handling (exact): bq is added on-device during the qT eviction;
bk shifts every score of a row equally -> softmax-invariant -> dropped;
bv passes through the softmax average exactly -> host adds bv @ Wp.T;
bp is added on the host.
"""

import os
import sys

if "/opt/trn_rl_repo" not in sys.path:
    sys.path.insert(0, "/opt/trn_rl_repo")

import numpy as np
import ml_dtypes

B, NQ, NK, C, H = 4, 1024, 2048, 1024, 16
D = C // H          # 64
HC = H // 2         # 8 heads per core
CO = HC * D         # 512 output dims per core
N_CORES = 8

_CACHE = {}


def _install_ntff_hook():
    """Register the axon NTFF profile hook (missing antenv.axon_hooks shim).
    Only needed when tracing; harmless otherwise."""
    import types

    if "antenv.axon_hooks" in sys.modules:
        return
    state = {"hook": None}
    mod = types.ModuleType("antenv.axon_hooks")
    mod.set_axon_ntff_profile_hook = lambda h: state.__setitem__("hook", h)
    mod.get_axon_ntff_profile_hook = lambda: state["hook"]
    sys.modules["antenv.axon_hooks"] = mod
    try:
        from trn_agent_boot.trn_boot import _ntff_profile_via_ctypes

        mod.set_axon_ntff_profile_hook(
            _ntff_profile_via_ctypes("/opt/axon/libaxon_pjrt.so")
        )
    except Exception:
        pass


def _build():
    import concourse.mybir as mybir
    import concourse.tile as tile
    from concourse import bacc

    F32 = mybir.dt.float32
    BF16 = mybir.dt.bfloat16
    Exp = mybir.ActivationFunctionType.Exp

    nc = bacc.Bacc("TRN2", target_bir_lowering=False, debug=False,
                   num_devices=N_CORES)

    def din(name, shape, dt=BF16):
        return nc.dram_tensor(name, shape, dt, kind="ExternalInput").ap()

    xT = din("xT", [C, NQ])            # x[b].T
    yT = din("yT", [C, NK])            # y[b].T
    m01T = din("m01T", [NK, NQ])       # keep=1 / masked=0, transposed
    wqT = din("wqT", [C, CO])          # (Wq[rows]*scale).T
    wkT = din("wkT", [C, CO])
    wvT = din("wvT", [C, CO])
    wpT = din("wpT", [CO, C])          # Wp[:, rows].T
    bqv = din("bq", [CO], mybir.dt.float32)   # scaled bq slice
    part = nc.dram_tensor("part", [NQ, C], BF16, kind="ExternalOutput").ap()

    LA = 3  # score -> attn-v lookahead (chunks in flight)

    with tile.TileContext(nc) as tc:
        with (
            tc.tile_pool(name="persist", bufs=1) as persist,
            tc.tile_pool(name="work_e", bufs=2) as pe_,
            tc.tile_pool(name="work_p", bufs=4) as pp_,
            tc.tile_pool(name="work_d", bufs=1) as pd_,
            tc.tile_pool(name="ps_proj", bufs=2, space="PSUM") as ps_proj,
            tc.tile_pool(name="ps_st", bufs=2, space="PSUM") as ps_st,
            tc.tile_pool(name="ps_out", bufs=1, space="PSUM") as ps_out,
        ):
            # ---- persistent tiles --------------------------------------
            x_sb = [persist.tile([128, NQ], BF16, tag=f"x{c}", name=f"x{c}")
                    for c in range(8)]
            y_sb = [persist.tile([128, NK], BF16, tag=f"y{c}", name=f"y{c}")
                    for c in range(8)]
            wq_sb = [persist.tile([128, CO], BF16, tag=f"wq{c}", name=f"wq{c}")
                     for c in range(8)]
            wk_sb = [persist.tile([128, CO], BF16, tag=f"wk{c}", name=f"wk{c}")
                     for c in range(8)]
            wv_sb = [persist.tile([128, CO], BF16, tag=f"wv{c}", name=f"wv{c}")
                     for c in range(8)]
            wp_sb = [persist.tile([128, C], BF16, tag=f"wp{i}", name=f"wp{i}")
                     for i in range(4)]
            m_sb = [persist.tile([128, NQ], BF16, tag=f"m{i}", name=f"m{i}")
                    for i in range(16)]
            kT_sb = [persist.tile([128, NK], BF16, tag=f"kT{i}", name=f"kT{i}")
                     for i in range(4)]
            qT_sb = [persist.tile([128, NQ], BF16, tag=f"qT{i}", name=f"qT{i}")
                     for i in range(4)]
            # v: 16 interleaved blocks [ones|v_0|ones|v_1|...]: block 2h =
            # ones, block 2h+1 = v_h[s,d]. Stationary for head h is the
            # contiguous pair [:, 2h:2h+2, :] = [ones | v_h] -> out rows
            # 0:64 denominator, 64:128 numerator. (The BIR weights AP must
            # be a single free dim, so the pair must be contiguous; the
            # denominator must land on partition base 0 because the custom-
            # DVE reciprocal requires matching in/out partition bases.)
            v_sb = [persist.tile([128, 16, D], BF16, tag=f"v{i}", name=f"v{i}")
                    for i in range(16)]
            outF_sb = [persist.tile([128, NQ], BF16, tag=f"oF{i}",
                                    name=f"oF{i}") for i in range(4)]
            bq_sb = [persist.tile([128, 1], F32, tag=f"bq{i}", name=f"bq{i}")
                     for i in range(4)]
            po032 = [persist.tile([128, C], BF16, tag=f"po032_{tb}",
                                  name=f"po032_{tb}") for tb in range(3, 8)]
            warm = persist.tile([1, 8], F32, tag="warm", name="warm")
            warm2 = persist.tile([1, 8], F32, tag="warm2", name="warm2")

            # ones blocks of v (written once); the dummy exp forces the ACT
            # exp-table load (~2.7us) to happen during the DMA lead-in
            for sc in range(16):
                nc.vector.memset(v_sb[sc][:, 0:16:2, :], 1.0)
            nc.vector.memset(warm[:], 0.0)
            nc.scalar.activation(warm2[:], warm[:], Exp)

            # ---- input DMAs, in first-needed order ---------------------
            # The first-score critical path (y s-cols 0:512, wk, x, wq, m0)
            # is spread across five engine queues so it lands in ~1/4 the
            # serial time; everything later streams on the sync queue in
            # consumption order (v units from iter ~2 need wv + leading y
            # columns; kT(0,1..3) needs the rest of y).
            for cc in range(8):
                nc.sync.dma_start(y_sb[cc][:, 0:512],
                                  yT[cc * 128:(cc + 1) * 128, 0:512])
                nc.scalar.dma_start(wk_sb[cc][:],
                                    wkT[cc * 128:(cc + 1) * 128, :])
                nc.gpsimd.dma_start(x_sb[cc][:],
                                    xT[cc * 128:(cc + 1) * 128, :])
                nc.scalar.dma_start(wq_sb[cc][:],
                                    wqT[cc * 128:(cc + 1) * 128, :])
            for ob in range(4):
                nc.sync.dma_start(bq_sb[ob][:],
                                  bqv[ob * 128:(ob + 1) * 128][:, None])
            for sc in range(3):
                nc.sync.dma_start(m_sb[sc][:],
                                  m01T[sc * 128:(sc + 1) * 128, :])
            for cc in range(8):
                nc.gpsimd.dma_start(wv_sb[cc][:],
                                    wvT[cc * 128:(cc + 1) * 128, :])
            # y tail per 512-col block so kT(0,sb) unblocks in order
            for sb in range(1, 4):
                for cc in range(8):
                    nc.sync.dma_start(
                        y_sb[cc][:, sb * 512:(sb + 1) * 512],
                        yT[cc * 128:(cc + 1) * 128, sb * 512:(sb + 1) * 512])
            for sc in range(3, 8):
                nc.scalar.dma_start(m_sb[sc][:],
                                    m01T[sc * 128:(sc + 1) * 128, :])
            for sc in range(8, 16):
                nc.sync.dma_start(m_sb[sc][:],
                                  m01T[sc * 128:(sc + 1) * 128, :])
            for ob in range(4):
                nc.sync.dma_start(wp_sb[ob][:], wpT[ob * 128:(ob + 1) * 128, :])

            # ---- PE unit emitters --------------------------------------
            # each projection unit is split into two "chunks" of 4 matmuls
            # so the filler can be paced finely inside the attention loop
            def _proj_chunks(make_stationary, moving, evict, name):
                box = {}

                def c1():
                    box["t"] = ps_proj.tile([128, 512], F32, tag="pps",
                                            name=name)
                    for cc in range(4):
                        nc.tensor.matmul(
                            box["t"][:], make_stationary(cc), moving(cc),
                            start=(cc == 0), stop=False,
                            skip_group_check=True,
                        )

                def c2():
                    for cc in range(4, 8):
                        nc.tensor.matmul(
                            box["t"][:], make_stationary(cc), moving(cc),
                            start=False, stop=(cc == 7),
                            skip_group_check=True,
                        )
                    evict(box["t"])

                return [c1, c2]

            def qproj_chunks(ob, tc2):
                return _proj_chunks(
                    lambda cc: wq_sb[cc][:, ob * 128:(ob + 1) * 128],
                    lambda cc: x_sb[cc][:, tc2 * 512:(tc2 + 1) * 512],
                    lambda t: nc.vector.tensor_scalar_add(
                        qT_sb[ob][:, tc2 * 512:(tc2 + 1) * 512],
                        t[:], bq_sb[ob][:]),
                    f"qps{ob}_{tc2}",
                )

            def kT_chunks(ob, sc4):
                return _proj_chunks(
                    lambda cc: wk_sb[cc][:, ob * 128:(ob + 1) * 128],
                    lambda cc: y_sb[cc][:, sc4 * 512:(sc4 + 1) * 512],
                    lambda t: nc.vector.tensor_copy(
                        kT_sb[ob][:, sc4 * 512:(sc4 + 1) * 512], t[:]),
                    f"kps{ob}_{sc4}",
                )

            def v_chunks(sc):
                return _proj_chunks(
                    lambda cc: y_sb[cc][:, sc * 128:(sc + 1) * 128],
                    lambda cc: wv_sb[cc][:],
                    lambda t: nc.vector.tensor_copy(v_sb[sc][:, 1:16:2, :],
                                                    t[:]),
                    f"vps{sc}",
                )

            def dummy_unit(tag):
                # keeps the PE issue queue deep in filler-less iterations so
                # the tensor engine holds its high p-state; result never read
                dps = ps_proj.tile([128, 512], F32, tag="pps", name=tag)
                nc.tensor.matmul(
                    dps[:], kT_sb[0][0:64, 0:128], qT_sb[0][0:64, 0:512],
                    start=True, stop=True, skip_group_check=True,
                )

            # out-projection pre-pass for token block tb: oc 0..2 staged to
            # SBUF bf16; legal filler any time after pass (2,1)'s normalize.
            def dpre_chunks(tb):
                def one(co):
                    def c():
                        t = ps_proj.tile([128, 512], F32, tag="pps",
                                         name=f"dpre{tb}_{co}")
                        for oc in range(3):
                            nc.tensor.matmul(
                                t[:],
                                outF_sb[oc][:, tb * 128:(tb + 1) * 128],
                                wp_sb[oc][:, co * 512:(co + 1) * 512],
                                start=(oc == 0), stop=(oc == 2),
                                skip_group_check=True,
                            )
                        nc.vector.tensor_copy(
                            po032[tb - 3][:, co * 512:(co + 1) * 512], t[:])
                    return c
                return [one(0), one(1)]

            # ---- startup: kT(ob0,s0..3-part) + qT(ob0) -----------------
            for c in kT_chunks(0, 0):
                c()
            for tc2 in range(2):
                for c in qproj_chunks(0, tc2):
                    c()

            # filler schedule.  Pass (0,0) uses an explicit per-iteration
            # list tuned to deadlines (v unit sc emitted by iter sc+2, one
            # iter before AV(sc) at iter sc+LA; kT(0,sb) before score chunk
            # 4sb) and DMA arrival (nothing in iters 0-1).  Other passes
            # pace a flat list evenly; empty iterations emit a dummy.
            def p00_filler():
                k1, k2, k3 = kT_chunks(0, 1), kT_chunks(0, 2), kT_chunks(0, 3)
                v = [v_chunks(sc) for sc in range(16)]
                return [
                    [],                                       # it0
                    [],                                       # it1
                    [v[0][0], v[0][1]],                       # it2
                    [v[1][0], v[1][1], k1[0]],                # it3
                    [k1[1], v[2][0], v[2][1]],                # it4
                    [v[3][0], v[3][1], v[4][0]],              # it5
                    [v[4][1], v[5][0], v[5][1]],              # it6
                    [v[6][0], v[6][1], k2[0]],                # it7
                    [k2[1], v[7][0], v[7][1]],                # it8
                    [v[8][0], v[8][1], v[9][0]],              # it9
                    [v[9][1], v[10][0], v[10][1]],            # it10
                    [v[11][0], v[11][1], k3[0]],              # it11
                    [k3[1], v[12][0], v[12][1]],              # it12
                    [v[13][0], v[13][1]],                     # it13
                    [v[14][0], v[14][1]],                     # it14
                    [v[15][0], v[15][1]],                     # it15
                ]

            def flat(units):
                return [c for u in units for c in u]

            filler = {
                (0, 0): p00_filler(),
                (0, 1): flat([qproj_chunks(1, 0), kT_chunks(1, 0),
                              qproj_chunks(1, 1), kT_chunks(1, 1),
                              kT_chunks(1, 2), kT_chunks(1, 3)]),
                (1, 0): flat([qproj_chunks(2, 0), kT_chunks(2, 0),
                              kT_chunks(2, 1)]),
                (1, 1): flat([qproj_chunks(2, 1), kT_chunks(2, 2),
                              kT_chunks(2, 3)]),
                (2, 0): flat([qproj_chunks(3, 0), kT_chunks(3, 0),
                              kT_chunks(3, 1)]),
                (2, 1): flat([qproj_chunks(3, 1), kT_chunks(3, 2),
                              kT_chunks(3, 3)]),
                (3, 0): flat([dpre_chunks(3), dpre_chunks(4), dpre_chunks(5)]),
                (3, 1): flat([dpre_chunks(6), dpre_chunks(7)]),
            }

            # ---- attention: heads sequential, pipelined chunks ---------
            for hp in range(4):
                for h2 in range(2):
                    h = 2 * hp + h2
                    p0 = h2 * 64
                    fl = filler[(hp, h2)]
                    explicit = bool(fl) and isinstance(fl[0], list)
                    n_chunks = 0 if explicit else len(fl)
                    popped = 0
                    outps = ps_out.tile([128, NQ], F32, tag="outps",
                                        name=f"outps{hp}_{h2}")
                    pts = {}
                    for it in range(16 + LA):
                        if it >= LA:
                            # attn-v first: its inputs (pt, v) are LA chunks
                            # old, so the PE never stalls entering the iter
                            sc = it - LA
                            pt = pts.pop(sc)
                            vst = v_sb[sc][:, 2 * h:2 * h + 2, :]
                            for tc2 in range(2):
                                nc.tensor.matmul(
                                    outps[:, tc2 * 512:(tc2 + 1) * 512],
                                    vst,
                                    pt[:, tc2 * 512:(tc2 + 1) * 512],
                                    start=(sc == 0), stop=(sc == 15),
                                    skip_group_check=True,
                                )
                        if it < 16:
                            sc = it
                            # PE filler ahead of this chunk's score matmuls
                            did = 0
                            if explicit:
                                for cch in fl[it]:
                                    cch()
                                    did += 1
                            else:
                                want = -(-n_chunks * (it + 1) // 16)  # ceil
                                while popped < want:
                                    fl[popped]()
                                    popped += 1
                                    did += 1
                            if did == 0 and it >= 2:
                                dummy_unit(f"dmy{hp}_{h2}_{it}")
                            stp = ps_st.tile([128, NQ], F32, tag="stp",
                                             name=f"stp{hp}_{h2}_{sc}")
                            for tc2 in range(2):
                                nc.tensor.matmul(
                                    stp[:, tc2 * 512:(tc2 + 1) * 512],
                                    kT_sb[hp][p0:p0 + 64,
                                              sc * 128:(sc + 1) * 128],
                                    qT_sb[hp][p0:p0 + 64,
                                              tc2 * 512:(tc2 + 1) * 512],
                                    start=True, stop=True,
                                    skip_group_check=True,
                                )
                            e = pe_.tile([128, NQ], BF16, tag="e")
                            nc.scalar.activation(e[:], stp[:], Exp)
                            pt = pp_.tile([128, NQ], BF16, tag="pt")
                            nc.vector.tensor_mul(pt[:], e[:], m_sb[sc][:])
                            pts[sc] = pt
                    # normalize: denominator rows 0:64 (x64 replicated),
                    # numerator rows 64:128; custom-DVE recip reads PSUM
                    rcp = pd_.tile([64, NQ], F32, tag="rcp")
                    nc.vector.reciprocal_approx_fast(rcp[:],
                                                     outps[0:64, :])
                    nc.vector.tensor_mul(
                        outF_sb[hp][p0:p0 + 64, :], outps[64:128, :], rcp[:])

            # ---- output projection -------------------------------------
            # tb0/tb1 pre-accumulate oc=0..2 while the final normalize
            # (which produces outF[3] rows 64:128) is still in flight, so
            # the PE keeps running through the attention->projection seam.
            with tc.tile_pool(name="proj", bufs=2) as prj:
                def d_mms(tb, pps, oc_lo, oc_hi):
                    for oc in range(oc_lo, oc_hi):
                        for co in range(2):
                            nc.tensor.matmul(
                                pps[:, co * 512:(co + 1) * 512],
                                outF_sb[oc][:, tb * 128:(tb + 1) * 128],
                                wp_sb[oc][:, co * 512:(co + 1) * 512],
                                start=(oc == 0), stop=(oc == 3),
                                skip_group_check=True,
                            )

                dma_eng = [nc.sync, nc.scalar, nc.gpsimd]

                pps_of = {}
                for tb in range(2):
                    pps_of[tb] = ps_st.tile([128, NQ], F32, tag="stp",
                                            name=f"pps{tb}")
                    d_mms(tb, pps_of[tb], 0, 3)
                pair2 = [ps_proj.tile([128, 512], F32, tag="pps",
                                      name=f"ppre{co}") for co in range(2)]
                for oc in range(3):
                    for co in range(2):
                        nc.tensor.matmul(
                            pair2[co][:],
                            outF_sb[oc][:, 2 * 128:3 * 128],
                            wp_sb[oc][:, co * 512:(co + 1) * 512],
                            start=(oc == 0), stop=False,
                            skip_group_check=True,
                        )
                # tail: oc3 matmuls burst on the PE; evictions go to ACT
                # (tb0-2 copies) and DVE/GpSimd (tb3-7 adds); the 8 output
                # DMAs round-robin over three queues
                for tb in range(2):
                    po = prj.tile([128, C], BF16, tag="po")
                    d_mms(tb, pps_of[tb], 3, 4)
                    nc.scalar.copy(po[:], pps_of[tb][:])
                    dma_eng[tb % 3].dma_start(
                        part[tb * 128:(tb + 1) * 128, :], po[:])
                po2 = prj.tile([128, C], BF16, tag="po")
                for co in range(2):
                    nc.tensor.matmul(
                        pair2[co][:],
                        outF_sb[3][:, 2 * 128:3 * 128],
                        wp_sb[3][:, co * 512:(co + 1) * 512],
                        start=False, stop=True,
                        skip_group_check=True,
                    )
                    nc.scalar.copy(po2[:, co * 512:(co + 1) * 512],
                                   pair2[co][:])
                dma_eng[2].dma_start(part[2 * 128:3 * 128, :], po2[:])
                for tb in range(3, 8):
                    po = prj.tile([128, C], BF16, tag="po")
                    for co in range(2):
                        t = ps_proj.tile([128, 512], F32, tag="pps",
                                         name=f"dfin{tb}_{co}")
                        nc.tensor.matmul(
                            t[:],
                            outF_sb[3][:, tb * 128:(tb + 1) * 128],
                            wp_sb[3][:, co * 512:(co + 1) * 512],
                            start=True, stop=True,
                            skip_group_check=True,
                        )
                        nc.vector.tensor_add(
                            po[:, co * 512:(co + 1) * 512], t[:],
                            po032[tb - 3][:, co * 512:(co + 1) * 512])
                    dummy_unit(f"dmyfin{tb}")
                    dma_eng[tb % 3].dma_start(
                        part[tb * 128:(tb + 1) * 128, :], po[:])

    nc.compile()
    return nc


def _get_nc():
    if "nc" not in _CACHE:
        _CACHE["nc"] = _build()
    return _CACHE["nc"]


def kernel(x, y, mask, Wq, bq, Wkv, bkv, Wp, bp):
    _install_ntff_hook()
    from concourse.bass_utils import run_bass_kernel_spmd

    x = np.asarray(x, dtype=np.float32)
    y = np.asarray(y, dtype=np.float32)
    mask = np.asarray(mask)
    Wq = np.asarray(Wq, dtype=np.float32)
    Wkv = np.asarray(Wkv, dtype=np.float32)
    Wp = np.asarray(Wp, dtype=np.float32)
    bq = np.asarray(bq, dtype=np.float32)
    bkv = np.asarray(bkv, dtype=np.float32)
    bp = np.asarray(bp, dtype=np.float32)

    scale = D ** -0.5
    bf16 = ml_dtypes.bfloat16
    xTs = [np.ascontiguousarray(x[b].T).astype(bf16) for b in range(B)]
    yTs = [np.ascontiguousarray(y[b].T).astype(bf16) for b in range(B)]
    m01Ts = [
        np.ascontiguousarray((~mask[b, 0]).T.astype(np.float32)).astype(bf16)
        for b in range(B)
    ]
    wqTs, wkTs, wvTs, wpTs, bqs = [], [], [], [], []
    for hg in range(2):
        rows = slice(hg * CO, hg * CO + CO)
        wqTs.append(np.ascontiguousarray((Wq[rows] * scale).T).astype(bf16))
        wkTs.append(np.ascontiguousarray(Wkv[rows].T).astype(bf16))
        wvTs.append(np.ascontiguousarray(
            Wkv[C + hg * CO: C + hg * CO + CO].T).astype(bf16))
        wpTs.append(np.ascontiguousarray(Wp[:, rows].T).astype(bf16))
        bqs.append(np.ascontiguousarray(bq[rows] * scale))

    in_maps = []
    for c in range(N_CORES):
        b, hg = divmod(c, 2)
        in_maps.append({
            "xT": xTs[b], "yT": yTs[b], "m01T": m01Ts[b],
            "wqT": wqTs[hg], "wkT": wkTs[hg], "wvT": wvTs[hg],
            "wpT": wpTs[hg], "bq": bqs[hg],
        })

    nc = _get_nc()
    trace = os.environ.get("CC_ATTN_TRACE", "") == "1"
    res = run_bass_kernel_spmd(nc, in_maps, core_ids=list(range(N_CORES)),
                               trace=trace)
    _CACHE["last_result"] = res

    # host gather: sum the two head-group partials per batch + exact bias folds
    bias = bkv[C:] @ Wp.T + bp  # v-bias passes through softmax exactly
    out = np.empty((B, NQ, C), dtype=np.float32)
    for b in range(B):
        out[b] = (res.results[2 * b]["part"].astype(np.float32)
                  + res.results[2 * b + 1]["part"].astype(np.float32) + bias)
    return out


# revision 22
# speedup vs baseline: 1.0243x; 1.0243x over previous
"""Cross-attention kernel for 8 Trainium2 NeuronCores (Bass/Tile, SPMD).

Problem: nn_CrossAttention (B=4, NQ=1024, NK=2048, C=1024, H=16, D=64), fp32.

Sharding: (batch x head-group) across the 8 cores. Core c handles batch
b = c//2 and heads h0 = (c%2)*8 .. h0+8 (column-parallel q/k/v projections,
row-parallel output projection). Each core emits a partial output
projection [NQ, C]; the host sums the two partials per batch (+ biases).

Device dataflow is fully "feature-major" (transposed): the host passes
x.T / y.T / W.T so every matmul contraction runs over the SBUF partition
axis with no on-device transposes:

  qT[o,t]  = sum_c wqT[c,o] * xT[c,t]          (o-blocks of 128 = 2 heads)
  kT[o,s]  = sum_c wkT[c,o] * yT[c,s]
  v[s,o]   = sum_c yT[c,s] * wvT[c,o]          (token-major)
  ST[s,t]  = sum_d kT_h[d,s-chunk] * qT_h[d,t]   per head (K=64)
  P[s,t]   = exp(ST) * mask01[s,t]               (ACT exp from PSUM, DVE mul)
  out_aug  = sum_s [ones | v_h]^T P  -> rows 0:64 denominator (x64
             replicated), rows 64:128 numerator
  outF     = out_aug[64:128] * recip(out_aug[0:64])   (custom-DVE recip
             reads PSUM directly; no partition broadcast / row extract)
  partial[t,co] = sum_o outF[o,t-block] * wpT[o,co]

All projections are bf16 (fp8 was tried: softmax score noise transfers
~1:1 into the output - no sqrt(N) washout - and e4m3's ~3% per-operand
quantization lands the end-to-end error at 3.2e-2 > the 2e-2 gate).

Schedule: fully software-pipelined. The attention loop processes the 8
heads sequentially; v-projection and the kT/qT projections for later head
pairs run as PE "filler" inside the chunk loop so the tensor engine stays
busy while the ACT engine drains the exp stream. LA=3 chunk lookahead
covers the exp+mask latency and the early-DMA arrival of v. Iterations
with no filler emit one dummy matmul - without it the PE micro-idles every
chunk in the ACT-bound passes and the HAM clock-gate drops the PE to 1.2
GHz (measured: ~70us of half-clock). Lead-in DMAs are spread across five
engine queues so the first-score critical path (~5MB of y/wk/x/wq/m0) does
not serialize on one queue. PSUM: 2 banks projections + 4 banks scores
(2 in flight) + 2 banks attn-out = 8.

Bias handling (exact): bq is added on-device during the qT eviction;
bk shifts every score of a row equally -> softmax-invariant -> dropped;
bv passes through the softmax average exactly -> host adds bv @ Wp.T;
bp is added on the host.
"""

import os
import sys

if "/opt/trn_rl_repo" not in sys.path:
    sys.path.insert(0, "/opt/trn_rl_repo")

import numpy as np
import ml_dtypes

B, NQ, NK, C, H = 4, 1024, 2048, 1024, 16
D = C // H          # 64
HC = H // 2         # 8 heads per core
CO = HC * D         # 512 output dims per core
N_CORES = 8

_CACHE = {}


def _install_ntff_hook():
    """Register the axon NTFF profile hook (missing antenv.axon_hooks shim).
    Only needed when tracing; harmless otherwise."""
    import types

    if "antenv.axon_hooks" in sys.modules:
        return
    state = {"hook": None}
    mod = types.ModuleType("antenv.axon_hooks")
    mod.set_axon_ntff_profile_hook = lambda h: state.__setitem__("hook", h)
    mod.get_axon_ntff_profile_hook = lambda: state["hook"]
    sys.modules["antenv.axon_hooks"] = mod
    try:
        from trn_agent_boot.trn_boot import _ntff_profile_via_ctypes

        mod.set_axon_ntff_profile_hook(
            _ntff_profile_via_ctypes("/opt/axon/libaxon_pjrt.so")
        )
    except Exception:
        pass


def _build():
    import concourse.mybir as mybir
    import concourse.tile as tile
    from concourse import bacc

    F32 = mybir.dt.float32
    BF16 = mybir.dt.bfloat16
    Exp = mybir.ActivationFunctionType.Exp

    nc = bacc.Bacc("TRN2", target_bir_lowering=False, debug=False,
                   num_devices=N_CORES)

    def din(name, shape, dt=BF16):
        return nc.dram_tensor(name, shape, dt, kind="ExternalInput").ap()

    xT = din("xT", [C, NQ])            # x[b].T
    yT = din("yT", [C, NK])            # y[b].T
    m01T = din("m01T", [NK, NQ])       # keep=1 / masked=0, transposed
    wqT = din("wqT", [C, CO])          # (Wq[rows]*scale).T
    wkT = din("wkT", [C, CO])
    wvT = din("wvT", [C, CO])
    wpT = din("wpT", [CO, C])          # Wp[:, rows].T
    bqv = din("bq", [CO], mybir.dt.float32)   # scaled bq slice
    part = nc.dram_tensor("part", [NQ, C], BF16, kind="ExternalOutput").ap()

    LA = 3  # score -> attn-v lookahead (chunks in flight)

    with tile.TileContext(nc) as tc:
        with (
            tc.tile_pool(name="persist", bufs=1) as persist,
            tc.tile_pool(name="work_p", bufs=4) as pp_,
            tc.tile_pool(name="work_d", bufs=1) as pd_,
            tc.tile_pool(name="ps_proj", bufs=2, space="PSUM") as ps_proj,
            tc.tile_pool(name="ps_st", bufs=2, space="PSUM") as ps_st,
            tc.tile_pool(name="ps_out", bufs=1, space="PSUM") as ps_out,
        ):
            # ---- persistent tiles --------------------------------------
            x_sb = [persist.tile([128, NQ], BF16, tag=f"x{c}", name=f"x{c}")
                    for c in range(8)]
            y_sb = [persist.tile([128, NK], BF16, tag=f"y{c}", name=f"y{c}")
                    for c in range(8)]
            wq_sb = [persist.tile([128, CO], BF16, tag=f"wq{c}", name=f"wq{c}")
                     for c in range(8)]
            wk_sb = [persist.tile([128, CO], BF16, tag=f"wk{c}", name=f"wk{c}")
                     for c in range(8)]
            wv_sb = [persist.tile([128, CO], BF16, tag=f"wv{c}", name=f"wv{c}")
                     for c in range(8)]
            wp_sb = [persist.tile([128, C], BF16, tag=f"wp{i}", name=f"wp{i}")
                     for i in range(4)]
            m_sb = [persist.tile([128, NQ], BF16, tag=f"m{i}", name=f"m{i}")
                    for i in range(16)]
            kT_sb = [persist.tile([128, NK], BF16, tag=f"kT{i}", name=f"kT{i}")
                     for i in range(4)]
            qT_sb = [persist.tile([128, NQ], BF16, tag=f"qT{i}", name=f"qT{i}")
                     for i in range(4)]
            # v: 16 interleaved blocks [ones|v_0|ones|v_1|...]: block 2h =
            # ones, block 2h+1 = v_h[s,d]. Stationary for head h is the
            # contiguous pair [:, 2h:2h+2, :] = [ones | v_h] -> out rows
            # 0:64 denominator, 64:128 numerator. (The BIR weights AP must
            # be a single free dim, so the pair must be contiguous; the
            # denominator must land on partition base 0 because the custom-
            # DVE reciprocal requires matching in/out partition bases.)
            v_sb = [persist.tile([128, 16, D], BF16, tag=f"v{i}", name=f"v{i}")
                    for i in range(16)]
            outF_sb = [persist.tile([128, NQ], BF16, tag=f"oF{i}",
                                    name=f"oF{i}") for i in range(4)]
            bq_sb = [persist.tile([128, 1], F32, tag=f"bq{i}", name=f"bq{i}")
                     for i in range(4)]
            po032 = [persist.tile([128, C], BF16, tag=f"po032_{tb}",
                                  name=f"po032_{tb}") for tb in range(2, 8)]
            warm = persist.tile([1, 8], F32, tag="warm", name="warm")
            warm2 = persist.tile([1, 8], F32, tag="warm2", name="warm2")
            # exp bias passed as our own zeroed AP: a float bias would lower
            # to a framework const-AP whose DMA queues behind all the input
            # DMAs - it blocked the first exp (and the whole score pipeline
            # behind the 2-buffer PSUM) until ~35us.
            zb = persist.tile([128, 1], F32, tag="zb", name="zb")

            # ones blocks of v (written once); the dummy exp forces the ACT
            # exp-table load (~2.7us) to happen during the DMA lead-in
            for sc in range(16):
                nc.vector.memset(v_sb[sc][:, 0:16:2, :], 1.0)
            nc.vector.memset(warm[:], 0.0)
            nc.vector.memset(zb[:], 0.0)
            nc.scalar.activation(warm2[:], warm[:], Exp, bias=zb[0:1, :])

            # ---- input DMAs, in first-needed order ---------------------
            # The first-score critical path (y s-cols 0:512, wk, x, wq, m0)
            # is spread across five engine queues so it lands in ~1/4 the
            # serial time; everything later streams on the sync queue in
            # consumption order (v units from iter ~2 need wv + leading y
            # columns; kT(0,1..3) needs the rest of y).
            for cc in range(8):
                nc.sync.dma_start(y_sb[cc][:, 0:512],
                                  yT[cc * 128:(cc + 1) * 128, 0:512])
                nc.scalar.dma_start(wk_sb[cc][:],
                                    wkT[cc * 128:(cc + 1) * 128, :])
                nc.gpsimd.dma_start(x_sb[cc][:],
                                    xT[cc * 128:(cc + 1) * 128, :])
                nc.scalar.dma_start(wq_sb[cc][:],
                                    wqT[cc * 128:(cc + 1) * 128, :])
            for ob in range(4):
                nc.sync.dma_start(bq_sb[ob][:],
                                  bqv[ob * 128:(ob + 1) * 128][:, None])
            for sc in range(3):
                nc.sync.dma_start(m_sb[sc][:],
                                  m01T[sc * 128:(sc + 1) * 128, :])
            for cc in range(8):
                nc.gpsimd.dma_start(wv_sb[cc][:],
                                    wvT[cc * 128:(cc + 1) * 128, :])
            # y tail per 512-col block so kT(0,sb) unblocks in order
            for sb in range(1, 4):
                for cc in range(8):
                    nc.sync.dma_start(
                        y_sb[cc][:, sb * 512:(sb + 1) * 512],
                        yT[cc * 128:(cc + 1) * 128, sb * 512:(sb + 1) * 512])
            for sc in range(3, 8):
                nc.scalar.dma_start(m_sb[sc][:],
                                    m01T[sc * 128:(sc + 1) * 128, :])
            for sc in range(8, 16):
                nc.sync.dma_start(m_sb[sc][:],
                                  m01T[sc * 128:(sc + 1) * 128, :])
            for ob in range(4):
                nc.sync.dma_start(wp_sb[ob][:], wpT[ob * 128:(ob + 1) * 128, :])

            # ---- PE unit emitters --------------------------------------
            # each projection unit is split into two "chunks" of 4 matmuls
            # so the filler can be paced finely inside the attention loop
            def _proj_chunks(make_stationary, moving, evict, name):
                box = {}

                def c1():
                    box["t"] = ps_proj.tile([128, 512], F32, tag="pps",
                                            name=name)
                    for cc in range(4):
                        nc.tensor.matmul(
                            box["t"][:], make_stationary(cc), moving(cc),
                            start=(cc == 0), stop=False,
                            skip_group_check=True,
                        )

                def c2():
                    for cc in range(4, 8):
                        nc.tensor.matmul(
                            box["t"][:], make_stationary(cc), moving(cc),
                            start=False, stop=(cc == 7),
                            skip_group_check=True,
                        )
                    evict(box["t"])

                return [c1, c2]

            def qproj_chunks(ob, tc2):
                return _proj_chunks(
                    lambda cc: wq_sb[cc][:, ob * 128:(ob + 1) * 128],
                    lambda cc: x_sb[cc][:, tc2 * 512:(tc2 + 1) * 512],
                    lambda t: nc.vector.tensor_scalar_add(
                        qT_sb[ob][:, tc2 * 512:(tc2 + 1) * 512],
                        t[:], bq_sb[ob][:]),
                    f"qps{ob}_{tc2}",
                )

            def kT_chunks(ob, sc4):
                return _proj_chunks(
                    lambda cc: wk_sb[cc][:, ob * 128:(ob + 1) * 128],
                    lambda cc: y_sb[cc][:, sc4 * 512:(sc4 + 1) * 512],
                    lambda t: nc.vector.tensor_copy(
                        kT_sb[ob][:, sc4 * 512:(sc4 + 1) * 512], t[:]),
                    f"kps{ob}_{sc4}",
                )

            def v_chunks(sc):
                return _proj_chunks(
                    lambda cc: y_sb[cc][:, sc * 128:(sc + 1) * 128],
                    lambda cc: wv_sb[cc][:],
                    lambda t: nc.vector.tensor_copy(v_sb[sc][:, 1:16:2, :],
                                                    t[:]),
                    f"vps{sc}",
                )

            def dummy_unit(tag):
                # keeps the PE issue queue deep in filler-less iterations so
                # the tensor engine holds its high p-state; result never read
                dps = ps_proj.tile([128, 512], F32, tag="pps", name=tag)
                nc.tensor.matmul(
                    dps[:], kT_sb[0][0:64, 0:128], qT_sb[0][0:64, 0:512],
                    start=True, stop=True, skip_group_check=True,
                )

            # out-projection pre-pass for token block tb: oc 0..2 staged to
            # SBUF bf16; legal filler any time after pass (2,1)'s normalize.
            def dpre_chunks(tb):
                def one(co):
                    def c():
                        t = ps_proj.tile([128, 512], F32, tag="pps",
                                         name=f"dpre{tb}_{co}")
                        for oc in range(3):
                            nc.tensor.matmul(
                                t[:],
                                outF_sb[oc][:, tb * 128:(tb + 1) * 128],
                                wp_sb[oc][:, co * 512:(co + 1) * 512],
                                start=(oc == 0), stop=(oc == 2),
                                skip_group_check=True,
                            )
                        nc.vector.tensor_copy(
                            po032[tb - 2][:, co * 512:(co + 1) * 512], t[:])
                    return c
                return [one(0), one(1)]

            # ---- startup: kT(ob0,s0..3-part) + qT(ob0) -----------------
            for c in kT_chunks(0, 0):
                c()
            for tc2 in range(2):
                for c in qproj_chunks(0, tc2):
                    c()

            # filler schedule.  Pass (0,0) uses an explicit per-iteration
            # list tuned to deadlines (v unit sc emitted by iter sc+2, one
            # iter before AV(sc) at iter sc+LA; kT(0,sb) before score chunk
            # 4sb) and DMA arrival (nothing in iters 0-1).  Other passes
            # pace a flat list evenly; empty iterations emit a dummy.
            def p00_filler():
                k1, k2, k3 = kT_chunks(0, 1), kT_chunks(0, 2), kT_chunks(0, 3)
                v = [v_chunks(sc) for sc in range(16)]
                return [
                    [],                                       # it0
                    [],                                       # it1
                    [v[0][0], v[0][1]],                       # it2
                    [v[1][0], v[1][1], k1[0]],                # it3
                    [k1[1], v[2][0], v[2][1]],                # it4
                    [v[3][0], v[3][1], v[4][0]],              # it5
                    [v[4][1], v[5][0], v[5][1]],              # it6
                    [v[6][0], v[6][1], k2[0]],                # it7
                    [k2[1], v[7][0], v[7][1]],                # it8
                    [v[8][0], v[8][1], v[9][0]],              # it9
                    [v[9][1], v[10][0], v[10][1]],            # it10
                    [v[11][0], v[11][1], k3[0]],              # it11
                    [k3[1], v[12][0], v[12][1]],              # it12
                    [v[13][0], v[13][1]],                     # it13
                    [v[14][0], v[14][1]],                     # it14
                    [v[15][0], v[15][1]],                     # it15
                ]

            def flat(units):
                return [c for u in units for c in u]

            filler = {
                (0, 0): p00_filler(),
                (0, 1): flat([qproj_chunks(1, 0), kT_chunks(1, 0),
                              qproj_chunks(1, 1), kT_chunks(1, 1),
                              kT_chunks(1, 2), kT_chunks(1, 3)]),
                (1, 0): flat([qproj_chunks(2, 0), kT_chunks(2, 0),
                              kT_chunks(2, 1)]),
                (1, 1): flat([qproj_chunks(2, 1), kT_chunks(2, 2),
                              kT_chunks(2, 3)]),
                (2, 0): flat([qproj_chunks(3, 0), kT_chunks(3, 0),
                              kT_chunks(3, 1)]),
                (2, 1): flat([qproj_chunks(3, 1), kT_chunks(3, 2),
                              kT_chunks(3, 3)]),
                (3, 0): flat([dpre_chunks(2), dpre_chunks(3), dpre_chunks(4)]),
                (3, 1): flat([dpre_chunks(5), dpre_chunks(6), dpre_chunks(7)]),
            }

            # ---- attention: heads sequential, pipelined chunks ---------
            for hp in range(4):
                for h2 in range(2):
                    h = 2 * hp + h2
                    p0 = h2 * 64
                    fl = filler[(hp, h2)]
                    explicit = bool(fl) and isinstance(fl[0], list)
                    n_chunks = 0 if explicit else len(fl)
                    popped = 0
                    outps = ps_out.tile([128, NQ], F32, tag="outps",
                                        name=f"outps{hp}_{h2}")
                    pts = {}
                    for it in range(16 + LA):
                        if it >= LA:
                            # attn-v first: its inputs (pt, v) are LA chunks
                            # old, so the PE never stalls entering the iter
                            sc = it - LA
                            pt = pts.pop(sc)
                            vst = v_sb[sc][:, 2 * h:2 * h + 2, :]
                            for tc2 in range(2):
                                nc.tensor.matmul(
                                    outps[:, tc2 * 512:(tc2 + 1) * 512],
                                    vst,
                                    pt[:, tc2 * 512:(tc2 + 1) * 512],
                                    start=(sc == 0), stop=(sc == 15),
                                    skip_group_check=True,
                                )
                        if it < 16:
                            sc = it
                            # PE filler ahead of this chunk's score matmuls
                            did = 0
                            if explicit:
                                for cch in fl[it]:
                                    cch()
                                    did += 1
                            else:
                                want = -(-n_chunks * (it + 1) // 16)  # ceil
                                while popped < want:
                                    fl[popped]()
                                    popped += 1
                                    did += 1
                            if did == 0 and it >= 2:
                                dummy_unit(f"dmy{hp}_{h2}_{it}")
                            stp = ps_st.tile([128, NQ], F32, tag="stp",
                                             name=f"stp{hp}_{h2}_{sc}")
                            for tc2 in range(2):
                                nc.tensor.matmul(
                                    stp[:, tc2 * 512:(tc2 + 1) * 512],
                                    kT_sb[hp][p0:p0 + 64,
                                              sc * 128:(sc + 1) * 128],
                                    qT_sb[hp][p0:p0 + 64,
                                              tc2 * 512:(tc2 + 1) * 512],
                                    start=True, stop=True,
                                    skip_group_check=True,
                                )
                            pt = pp_.tile([128, NQ], BF16, tag="pt")
                            nc.scalar.activation(pt[:], stp[:], Exp,
                                                 bias=zb[:])
                            nc.vector.tensor_mul(pt[:], pt[:], m_sb[sc][:])
                            pts[sc] = pt
                    # normalize: denominator rows 0:64 (x64 replicated),
                    # numerator rows 64:128; custom-DVE recip reads PSUM
                    rcp = pd_.tile([64, NQ], F32, tag="rcp")
                    nc.vector.reciprocal_approx_fast(rcp[:],
                                                     outps[0:64, :])
                    nc.vector.tensor_mul(
                        outF_sb[hp][p0:p0 + 64, :], outps[64:128, :], rcp[:])

            # ---- output projection -------------------------------------
            # tb0/tb1 pre-accumulate oc=0..2 while the final normalize
            # (which produces outF[3] rows 64:128) is still in flight, so
            # the PE keeps running through the attention->projection seam.
            with tc.tile_pool(name="proj", bufs=2) as prj:
                def d_mms(tb, pps, oc_lo, oc_hi):
                    for oc in range(oc_lo, oc_hi):
                        for co in range(2):
                            nc.tensor.matmul(
                                pps[:, co * 512:(co + 1) * 512],
                                outF_sb[oc][:, tb * 128:(tb + 1) * 128],
                                wp_sb[oc][:, co * 512:(co + 1) * 512],
                                start=(oc == 0), stop=(oc == 3),
                                skip_group_check=True,
                            )

                dma_eng = [nc.sync, nc.scalar]

                pps_of = {}
                for tb in range(2):
                    pps_of[tb] = ps_st.tile([128, NQ], F32, tag="stp",
                                            name=f"pps{tb}")
                    d_mms(tb, pps_of[tb], 0, 3)
                # tail: oc3 matmuls burst on the PE; tb0/tb1 evict via ACT
                # copies, tb2-7 add in place into their staged po032 tiles
                # on DVE; the 8 output DMAs alternate over two queues
                # (gpsimd descriptor generation is too slow for the tail)
                for tb in range(2):
                    po = prj.tile([128, C], BF16, tag="po")
                    d_mms(tb, pps_of[tb], 3, 4)
                    nc.scalar.copy(po[:], pps_of[tb][:])
                    dma_eng[tb % 2].dma_start(
                        part[tb * 128:(tb + 1) * 128, :], po[:])
                for tb in range(2, 8):
                    for co in range(2):
                        t = ps_proj.tile([128, 512], F32, tag="pps",
                                         name=f"dfin{tb}_{co}")
                        nc.tensor.matmul(
                            t[:],
                            outF_sb[3][:, tb * 128:(tb + 1) * 128],
                            wp_sb[3][:, co * 512:(co + 1) * 512],
                            start=True, stop=True,
                            skip_group_check=True,
                        )
                        nc.vector.tensor_add(
                            po032[tb - 2][:, co * 512:(co + 1) * 512], t[:],
                            po032[tb - 2][:, co * 512:(co + 1) * 512])
                    dummy_unit(f"dmyfin{tb}")
                    dma_eng[tb % 2].dma_start(
                        part[tb * 128:(tb + 1) * 128, :], po032[tb - 2][:])

    nc.compile()
    return nc


def _get_nc():
    if "nc" not in _CACHE:
        _CACHE["nc"] = _build()
    return _CACHE["nc"]


def kernel(x, y, mask, Wq, bq, Wkv, bkv, Wp, bp):
    _install_ntff_hook()
    from concourse.bass_utils import run_bass_kernel_spmd

    x = np.asarray(x, dtype=np.float32)
    y = np.asarray(y, dtype=np.float32)
    mask = np.asarray(mask)
    Wq = np.asarray(Wq, dtype=np.float32)
    Wkv = np.asarray(Wkv, dtype=np.float32)
    Wp = np.asarray(Wp, dtype=np.float32)
    bq = np.asarray(bq, dtype=np.float32)
    bkv = np.asarray(bkv, dtype=np.float32)
    bp = np.asarray(bp, dtype=np.float32)

    scale = D ** -0.5
    bf16 = ml_dtypes.bfloat16
    xTs = [np.ascontiguousarray(x[b].T).astype(bf16) for b in range(B)]
    yTs = [np.ascontiguousarray(y[b].T).astype(bf16) for b in range(B)]
    m01Ts = [
        np.ascontiguousarray((~mask[b, 0]).T.astype(np.float32)).astype(bf16)
        for b in range(B)
    ]
    wqTs, wkTs, wvTs, wpTs, bqs = [], [], [], [], []
    for hg in range(2):
        rows = slice(hg * CO, hg * CO + CO)
        wqTs.append(np.ascontiguousarray((Wq[rows] * scale).T).astype(bf16))
        wkTs.append(np.ascontiguousarray(Wkv[rows].T).astype(bf16))
        wvTs.append(np.ascontiguousarray(
            Wkv[C + hg * CO: C + hg * CO + CO].T).astype(bf16))
        wpTs.append(np.ascontiguousarray(Wp[:, rows].T).astype(bf16))
        bqs.append(np.ascontiguousarray(bq[rows] * scale))

    in_maps = []
    for c in range(N_CORES):
        b, hg = divmod(c, 2)
        in_maps.append({
            "xT": xTs[b], "yT": yTs[b], "m01T": m01Ts[b],
            "wqT": wqTs[hg], "wkT": wkTs[hg], "wvT": wvTs[hg],
            "wpT": wpTs[hg], "bq": bqs[hg],
        })

    nc = _get_nc()
    trace = os.environ.get("CC_ATTN_TRACE", "") == "1"
    res = run_bass_kernel_spmd(nc, in_maps, core_ids=list(range(N_CORES)),
                               trace=trace)
    _CACHE["last_result"] = res

    # host gather: sum the two head-group partials per batch + exact bias folds
    bias = bkv[C:] @ Wp.T + bp  # v-bias passes through softmax exactly
    out = np.empty((B, NQ, C), dtype=np.float32)
    for b in range(B):
        out[b] = (res.results[2 * b]["part"].astype(np.float32)
                  + res.results[2 * b + 1]["part"].astype(np.float32) + bias)
    return out


# revision 24
# speedup vs baseline: 1.0420x; 1.0172x over previous
"""Cross-attention kernel for 8 Trainium2 NeuronCores (Bass/Tile, SPMD).

Problem: nn_CrossAttention (B=4, NQ=1024, NK=2048, C=1024, H=16, D=64), fp32.

Sharding: (batch x head-group) across the 8 cores. Core c handles batch
b = c//2 and heads h0 = (c%2)*8 .. h0+8 (column-parallel q/k/v projections,
row-parallel output projection). Each core emits a partial output
projection [NQ, C]; the host sums the two partials per batch (+ biases).

Device dataflow is fully "feature-major" (transposed): the host passes
x.T / y.T / W.T so every matmul contraction runs over the SBUF partition
axis with no on-device transposes:

  qT[o,t]  = sum_c wqT[c,o] * xT[c,t]          (o-blocks of 128 = 2 heads)
  kT[o,s]  = sum_c wkT[c,o] * yT[c,s]
  v[s,o]   = sum_c yT[c,s] * wvT[c,o]          (token-major)
  ST[s,t]  = sum_d kT_h[d,s-chunk] * qT_h[d,t]   per head (K=64)
  P[s,t]   = exp(ST) * mask01[s,t]               (ACT exp from PSUM, DVE mul)
  out_aug  = sum_s [ones | v_h]^T P  -> rows 0:64 denominator (x64
             replicated), rows 64:128 numerator
  outF     = out_aug[64:128] * recip(out_aug[0:64])   (custom-DVE recip
             reads PSUM directly; no partition broadcast / row extract)
  partial[t,co] = sum_o outF[o,t-block] * wpT[o,co]

All projections are bf16 (fp8 was tried: softmax score noise transfers
~1:1 into the output - no sqrt(N) washout - and e4m3's ~3% per-operand
quantization lands the end-to-end error at 3.2e-2 > the 2e-2 gate).

Schedule: fully software-pipelined. The attention loop processes the 8
heads sequentially; v-projection and the kT/qT projections for later head
pairs run as PE "filler" inside the chunk loop so the tensor engine stays
busy while the ACT engine drains the exp stream. LA=3 chunk lookahead
covers the exp+mask latency and the early-DMA arrival of v. Iterations
with no filler emit one dummy matmul - without it the PE micro-idles every
chunk in the ACT-bound passes and the HAM clock-gate drops the PE to 1.2
GHz (measured: ~70us of half-clock). Lead-in DMAs are spread across five
engine queues so the first-score critical path (~5MB of y/wk/x/wq/m0) does
not serialize on one queue. PSUM: 2 banks projections + 4 banks scores
(2 in flight) + 2 banks attn-out = 8.

Bias handling (exact): bq is added on-device during the qT eviction;
bk shifts every score of a row equally -> softmax-invariant -> dropped;
bv passes through the softmax average exactly -> host adds bv @ Wp.T;
bp is added on the host.
"""

import os
import sys

if "/opt/trn_rl_repo" not in sys.path:
    sys.path.insert(0, "/opt/trn_rl_repo")

import numpy as np
import ml_dtypes

B, NQ, NK, C, H = 4, 1024, 2048, 1024, 16
D = C // H          # 64
HC = H // 2         # 8 heads per core
CO = HC * D         # 512 output dims per core
N_CORES = 8

_CACHE = {}


def _install_ntff_hook():
    """Register the axon NTFF profile hook (missing antenv.axon_hooks shim).
    Only needed when tracing; harmless otherwise."""
    import types

    if "antenv.axon_hooks" in sys.modules:
        return
    state = {"hook": None}
    mod = types.ModuleType("antenv.axon_hooks")
    mod.set_axon_ntff_profile_hook = lambda h: state.__setitem__("hook", h)
    mod.get_axon_ntff_profile_hook = lambda: state["hook"]
    sys.modules["antenv.axon_hooks"] = mod
    try:
        from trn_agent_boot.trn_boot import _ntff_profile_via_ctypes

        mod.set_axon_ntff_profile_hook(
            _ntff_profile_via_ctypes("/opt/axon/libaxon_pjrt.so")
        )
    except Exception:
        pass


def _build():
    import concourse.mybir as mybir
    import concourse.tile as tile
    from concourse import bacc

    F32 = mybir.dt.float32
    BF16 = mybir.dt.bfloat16
    Exp = mybir.ActivationFunctionType.Exp

    nc = bacc.Bacc("TRN2", target_bir_lowering=False, debug=False,
                   num_devices=N_CORES)

    def din(name, shape, dt=BF16):
        return nc.dram_tensor(name, shape, dt, kind="ExternalInput").ap()

    xT = din("xT", [C, NQ])            # x[b].T
    yT = din("yT", [C, NK])            # y[b].T
    m01T = din("m01T", [NK, NQ])       # keep=1 / masked=0, transposed
    wqT = din("wqT", [C, CO])          # (Wq[rows]*scale).T
    wkT = din("wkT", [C, CO])
    wvT = din("wvT", [C, CO])
    wpT = din("wpT", [CO, C])          # Wp[:, rows].T
    bqv = din("bq", [CO], mybir.dt.float32)   # scaled bq slice
    part = nc.dram_tensor("part", [NQ, C], BF16, kind="ExternalOutput").ap()

    LA = 3  # score -> attn-v lookahead (chunks in flight)

    with tile.TileContext(nc) as tc:
        with (
            tc.tile_pool(name="persist", bufs=1) as persist,
            tc.tile_pool(name="work_p", bufs=4) as pp_,
            tc.tile_pool(name="work_d", bufs=1) as pd_,
            tc.tile_pool(name="ps_proj", bufs=2, space="PSUM") as ps_proj,
            tc.tile_pool(name="ps_st", bufs=2, space="PSUM") as ps_st,
            tc.tile_pool(name="ps_out", bufs=1, space="PSUM") as ps_out,
        ):
            # ---- persistent tiles --------------------------------------
            x_sb = [persist.tile([128, NQ], BF16, tag=f"x{c}", name=f"x{c}")
                    for c in range(8)]
            y_sb = [persist.tile([128, NK], BF16, tag=f"y{c}", name=f"y{c}")
                    for c in range(8)]
            wq_sb = [persist.tile([128, CO], BF16, tag=f"wq{c}", name=f"wq{c}")
                     for c in range(8)]
            wk_sb = [persist.tile([128, CO], BF16, tag=f"wk{c}", name=f"wk{c}")
                     for c in range(8)]
            wv_sb = [persist.tile([128, CO], BF16, tag=f"wv{c}", name=f"wv{c}")
                     for c in range(8)]
            wp_sb = [persist.tile([128, C], BF16, tag=f"wp{i}", name=f"wp{i}")
                     for i in range(4)]
            m_sb = [persist.tile([128, NQ], BF16, tag=f"m{i}", name=f"m{i}")
                    for i in range(16)]
            kT_sb = [persist.tile([128, NK], BF16, tag=f"kT{i}", name=f"kT{i}")
                     for i in range(4)]
            qT_sb = [persist.tile([128, NQ], BF16, tag=f"qT{i}", name=f"qT{i}")
                     for i in range(4)]
            # v: 16 interleaved blocks [ones|v_0|ones|v_1|...]: block 2h =
            # ones, block 2h+1 = v_h[s,d]. Stationary for head h is the
            # contiguous pair [:, 2h:2h+2, :] = [ones | v_h] -> out rows
            # 0:64 denominator, 64:128 numerator. (The BIR weights AP must
            # be a single free dim, so the pair must be contiguous; the
            # denominator must land on partition base 0 because the custom-
            # DVE reciprocal requires matching in/out partition bases.)
            v_sb = [persist.tile([128, 16, D], BF16, tag=f"v{i}", name=f"v{i}")
                    for i in range(16)]
            outF_sb = [persist.tile([128, NQ], BF16, tag=f"oF{i}",
                                    name=f"oF{i}") for i in range(4)]
            bq_sb = [persist.tile([128, 1], F32, tag=f"bq{i}", name=f"bq{i}")
                     for i in range(4)]
            po032 = [persist.tile([128, C], BF16, tag=f"po032_{tb}",
                                  name=f"po032_{tb}") for tb in range(2, 8)]
            warm = persist.tile([1, 8], F32, tag="warm", name="warm")
            warm2 = persist.tile([1, 8], F32, tag="warm2", name="warm2")
            # exp bias passed as our own zeroed AP: a float bias would lower
            # to a framework const-AP whose DMA queues behind all the input
            # DMAs - it blocked the first exp (and the whole score pipeline
            # behind the 2-buffer PSUM) until ~35us.
            zb = persist.tile([128, 1], F32, tag="zb", name="zb")

            # dummy exp first: forces the ACT exp-table load (~2.7us) at
            # t~1us. The ones blocks of v go on the scalar queue too - ACT
            # is idle until the first real exp, while the vector queue must
            # stay clear for the first kT/qT evictions.
            nc.vector.memset(warm[:], 0.0)
            nc.vector.memset(zb[:], 0.0)
            nc.scalar.activation(warm2[:], warm[:], Exp, bias=zb[0:1, :])

            # ---- input DMAs, in first-needed order ---------------------
            # The first-score critical path (y s-cols 0:512, wk, x, wq, m0)
            # is spread across five engine queues so it lands in ~1/4 the
            # serial time; everything later streams on the sync queue in
            # consumption order (v units from iter ~2 need wv + leading y
            # columns; kT(0,1..3) needs the rest of y).
            for cc in range(8):
                nc.sync.dma_start(y_sb[cc][:, 0:512],
                                  yT[cc * 128:(cc + 1) * 128, 0:512])
                nc.gpsimd.dma_start(wk_sb[cc][:],
                                    wkT[cc * 128:(cc + 1) * 128, :])
            for ob in range(4):
                nc.sync.dma_start(bq_sb[ob][:],
                                  bqv[ob * 128:(ob + 1) * 128][:, None])
            for cc in range(8):
                nc.sync.dma_start(x_sb[cc][:, 0:512],
                                  xT[cc * 128:(cc + 1) * 128, 0:512])
                nc.gpsimd.dma_start(wq_sb[cc][:],
                                    wqT[cc * 128:(cc + 1) * 128, :])
            for sc in range(3):
                nc.sync.dma_start(m_sb[sc][:],
                                  m01T[sc * 128:(sc + 1) * 128, :])
            for cc in range(8):
                nc.gpsimd.dma_start(x_sb[cc][:, 512:NQ],
                                    xT[cc * 128:(cc + 1) * 128, 512:NQ])
            for cc in range(8):
                nc.gpsimd.dma_start(wv_sb[cc][:],
                                    wvT[cc * 128:(cc + 1) * 128, :])
            # y tail per 512-col block so kT(0,sb) unblocks in order
            for sb in range(1, 4):
                for cc in range(8):
                    nc.sync.dma_start(
                        y_sb[cc][:, sb * 512:(sb + 1) * 512],
                        yT[cc * 128:(cc + 1) * 128, sb * 512:(sb + 1) * 512])
            for sc in range(3, 8):
                nc.gpsimd.dma_start(m_sb[sc][:],
                                    m01T[sc * 128:(sc + 1) * 128, :])
            for sc in range(8, 16):
                nc.sync.dma_start(m_sb[sc][:],
                                  m01T[sc * 128:(sc + 1) * 128, :])
            for ob in range(4):
                nc.sync.dma_start(wp_sb[ob][:], wpT[ob * 128:(ob + 1) * 128, :])

            # ---- PE unit emitters --------------------------------------
            # each projection unit is split into two "chunks" of 4 matmuls
            # so the filler can be paced finely inside the attention loop
            def _proj_chunks(make_stationary, moving, evict, name):
                box = {}

                def c1():
                    box["t"] = ps_proj.tile([128, 512], F32, tag="pps",
                                            name=name)
                    for cc in range(4):
                        nc.tensor.matmul(
                            box["t"][:], make_stationary(cc), moving(cc),
                            start=(cc == 0), stop=False,
                            skip_group_check=True,
                        )

                def c2():
                    for cc in range(4, 8):
                        nc.tensor.matmul(
                            box["t"][:], make_stationary(cc), moving(cc),
                            start=False, stop=(cc == 7),
                            skip_group_check=True,
                        )
                    evict(box["t"])

                return [c1, c2]

            def qproj_chunks(ob, tc2):
                return _proj_chunks(
                    lambda cc: wq_sb[cc][:, ob * 128:(ob + 1) * 128],
                    lambda cc: x_sb[cc][:, tc2 * 512:(tc2 + 1) * 512],
                    lambda t: nc.vector.tensor_scalar_add(
                        qT_sb[ob][:, tc2 * 512:(tc2 + 1) * 512],
                        t[:], bq_sb[ob][:]),
                    f"qps{ob}_{tc2}",
                )

            def kT_chunks(ob, sc4):
                return _proj_chunks(
                    lambda cc: wk_sb[cc][:, ob * 128:(ob + 1) * 128],
                    lambda cc: y_sb[cc][:, sc4 * 512:(sc4 + 1) * 512],
                    lambda t: nc.vector.tensor_copy(
                        kT_sb[ob][:, sc4 * 512:(sc4 + 1) * 512], t[:]),
                    f"kps{ob}_{sc4}",
                )

            def v_chunks(sc):
                return _proj_chunks(
                    lambda cc: y_sb[cc][:, sc * 128:(sc + 1) * 128],
                    lambda cc: wv_sb[cc][:],
                    lambda t: nc.vector.tensor_copy(v_sb[sc][:, 1:16:2, :],
                                                    t[:]),
                    f"vps{sc}",
                )

            def dummy_unit(tag):
                # keeps the PE issue queue deep in filler-less iterations so
                # the tensor engine holds its high p-state; result never read
                dps = ps_proj.tile([128, 512], F32, tag="pps", name=tag)
                nc.tensor.matmul(
                    dps[:], kT_sb[0][0:64, 0:128], qT_sb[0][0:64, 0:512],
                    start=True, stop=True, skip_group_check=True,
                )

            # out-projection pre-pass for token block tb: oc 0..2 staged to
            # SBUF bf16; legal filler any time after pass (2,1)'s normalize.
            def dpre_chunks(tb):
                def one(co):
                    def c():
                        t = ps_proj.tile([128, 512], F32, tag="pps",
                                         name=f"dpre{tb}_{co}")
                        for oc in range(3):
                            nc.tensor.matmul(
                                t[:],
                                outF_sb[oc][:, tb * 128:(tb + 1) * 128],
                                wp_sb[oc][:, co * 512:(co + 1) * 512],
                                start=(oc == 0), stop=(oc == 2),
                                skip_group_check=True,
                            )
                        nc.vector.tensor_copy(
                            po032[tb - 2][:, co * 512:(co + 1) * 512], t[:])
                    return c
                return [one(0), one(1)]

            # ---- startup: kT(ob0,s0..3-part) + qT(ob0) -----------------
            for c in kT_chunks(0, 0):
                c()
            for tc2 in range(2):
                for c in qproj_chunks(0, tc2):
                    c()
            # v ones blocks: emitted after the startup units so the vector
            # queue runs the first kT/qT evictions before these 16 memsets
            for sc in range(16):
                nc.vector.memset(v_sb[sc][:, 0:16:2, :], 1.0)

            # filler schedule.  Pass (0,0) uses an explicit per-iteration
            # list tuned to deadlines (v unit sc emitted by iter sc+2, one
            # iter before AV(sc) at iter sc+LA; kT(0,sb) before score chunk
            # 4sb) and DMA arrival (nothing in iters 0-1).  Other passes
            # pace a flat list evenly; empty iterations emit a dummy.
            def p00_filler():
                k1, k2, k3 = kT_chunks(0, 1), kT_chunks(0, 2), kT_chunks(0, 3)
                v = [v_chunks(sc) for sc in range(16)]
                return [
                    [],                                       # it0
                    [],                                       # it1
                    [v[0][0], v[0][1]],                       # it2
                    [v[1][0], v[1][1], k1[0]],                # it3
                    [k1[1], v[2][0], v[2][1]],                # it4
                    [v[3][0], v[3][1], v[4][0]],              # it5
                    [v[4][1], v[5][0], v[5][1]],              # it6
                    [v[6][0], v[6][1], k2[0]],                # it7
                    [k2[1], v[7][0], v[7][1]],                # it8
                    [v[8][0], v[8][1], v[9][0]],              # it9
                    [v[9][1], v[10][0], v[10][1]],            # it10
                    [v[11][0], v[11][1], k3[0]],              # it11
                    [k3[1], v[12][0], v[12][1]],              # it12
                    [v[13][0], v[13][1]],                     # it13
                    [v[14][0], v[14][1]],                     # it14
                    [v[15][0], v[15][1]],                     # it15
                ]

            def flat(units):
                return [c for u in units for c in u]

            filler = {
                (0, 0): p00_filler(),
                (0, 1): flat([qproj_chunks(1, 0), kT_chunks(1, 0),
                              qproj_chunks(1, 1), kT_chunks(1, 1),
                              kT_chunks(1, 2), kT_chunks(1, 3)]),
                (1, 0): flat([qproj_chunks(2, 0), kT_chunks(2, 0),
                              kT_chunks(2, 1)]),
                (1, 1): flat([qproj_chunks(2, 1), kT_chunks(2, 2),
                              kT_chunks(2, 3)]),
                (2, 0): flat([qproj_chunks(3, 0), kT_chunks(3, 0),
                              kT_chunks(3, 1)]),
                (2, 1): flat([qproj_chunks(3, 1), kT_chunks(3, 2),
                              kT_chunks(3, 3)]),
                (3, 0): flat([dpre_chunks(2), dpre_chunks(3), dpre_chunks(4)]),
                (3, 1): flat([dpre_chunks(5), dpre_chunks(6), dpre_chunks(7)]),
            }

            # ---- attention: heads sequential, pipelined chunks ---------
            for hp in range(4):
                for h2 in range(2):
                    h = 2 * hp + h2
                    p0 = h2 * 64
                    fl = filler[(hp, h2)]
                    explicit = bool(fl) and isinstance(fl[0], list)
                    n_chunks = 0 if explicit else len(fl)
                    popped = 0
                    outps = ps_out.tile([128, NQ], F32, tag="outps",
                                        name=f"outps{hp}_{h2}")
                    pts = {}
                    for it in range(16 + LA):
                        if it >= LA:
                            # attn-v first: its inputs (pt, v) are LA chunks
                            # old, so the PE never stalls entering the iter
                            sc = it - LA
                            pt = pts.pop(sc)
                            vst = v_sb[sc][:, 2 * h:2 * h + 2, :]
                            for tc2 in range(2):
                                nc.tensor.matmul(
                                    outps[:, tc2 * 512:(tc2 + 1) * 512],
                                    vst,
                                    pt[:, tc2 * 512:(tc2 + 1) * 512],
                                    start=(sc == 0), stop=(sc == 15),
                                    skip_group_check=True,
                                )
                        if it < 16:
                            sc = it
                            # PE filler ahead of this chunk's score matmuls
                            did = 0
                            if explicit:
                                for cch in fl[it]:
                                    cch()
                                    did += 1
                            else:
                                want = -(-n_chunks * (it + 1) // 16)  # ceil
                                while popped < want:
                                    fl[popped]()
                                    popped += 1
                                    did += 1
                            if did == 0 and it >= 2:
                                dummy_unit(f"dmy{hp}_{h2}_{it}")
                            stp = ps_st.tile([128, NQ], F32, tag="stp",
                                             name=f"stp{hp}_{h2}_{sc}")
                            for tc2 in range(2):
                                nc.tensor.matmul(
                                    stp[:, tc2 * 512:(tc2 + 1) * 512],
                                    kT_sb[hp][p0:p0 + 64,
                                              sc * 128:(sc + 1) * 128],
                                    qT_sb[hp][p0:p0 + 64,
                                              tc2 * 512:(tc2 + 1) * 512],
                                    start=True, stop=True,
                                    skip_group_check=True,
                                )
                            pt = pp_.tile([128, NQ], BF16, tag="pt")
                            nc.scalar.activation(pt[:], stp[:], Exp,
                                                 bias=zb[:])
                            nc.vector.tensor_mul(pt[:], pt[:], m_sb[sc][:])
                            pts[sc] = pt
                    # normalize: denominator rows 0:64 (x64 replicated),
                    # numerator rows 64:128; custom-DVE recip reads PSUM
                    rcp = pd_.tile([64, NQ], F32, tag="rcp")
                    nc.vector.reciprocal_approx_fast(rcp[:],
                                                     outps[0:64, :])
                    nc.vector.tensor_mul(
                        outF_sb[hp][p0:p0 + 64, :], outps[64:128, :], rcp[:])

            # ---- output projection -------------------------------------
            # tb0/tb1 pre-accumulate oc=0..2 while the final normalize
            # (which produces outF[3] rows 64:128) is still in flight, so
            # the PE keeps running through the attention->projection seam.
            with tc.tile_pool(name="proj", bufs=2) as prj:
                def d_mms(tb, pps, oc_lo, oc_hi):
                    for oc in range(oc_lo, oc_hi):
                        for co in range(2):
                            nc.tensor.matmul(
                                pps[:, co * 512:(co + 1) * 512],
                                outF_sb[oc][:, tb * 128:(tb + 1) * 128],
                                wp_sb[oc][:, co * 512:(co + 1) * 512],
                                start=(oc == 0), stop=(oc == 3),
                                skip_group_check=True,
                            )

                dma_eng = [nc.sync, nc.scalar]

                pps_of = {}
                for tb in range(2):
                    pps_of[tb] = ps_st.tile([128, NQ], F32, tag="stp",
                                            name=f"pps{tb}")
                    d_mms(tb, pps_of[tb], 0, 3)
                # tail: oc3 matmuls burst on the PE; tb0/tb1 evict via ACT
                # copies, tb2-7 add in place into their staged po032 tiles
                # on DVE; the 8 output DMAs alternate over two queues
                # (gpsimd descriptor generation is too slow for the tail)
                for tb in range(2):
                    po = prj.tile([128, C], BF16, tag="po")
                    d_mms(tb, pps_of[tb], 3, 4)
                    nc.scalar.copy(po[:], pps_of[tb][:])
                    dma_eng[tb % 2].dma_start(
                        part[tb * 128:(tb + 1) * 128, :], po[:])
                for tb in range(2, 8):
                    for co in range(2):
                        t = ps_proj.tile([128, 512], F32, tag="pps",
                                         name=f"dfin{tb}_{co}")
                        nc.tensor.matmul(
                            t[:],
                            outF_sb[3][:, tb * 128:(tb + 1) * 128],
                            wp_sb[3][:, co * 512:(co + 1) * 512],
                            start=True, stop=True,
                            skip_group_check=True,
                        )
                        nc.vector.tensor_add(
                            po032[tb - 2][:, co * 512:(co + 1) * 512], t[:],
                            po032[tb - 2][:, co * 512:(co + 1) * 512])
                    dummy_unit(f"dmyfin{tb}")
                    dma_eng[tb % 2].dma_start(
                        part[tb * 128:(tb + 1) * 128, :], po032[tb - 2][:])

    nc.compile()
    return nc


def _get_nc():
    if "nc" not in _CACHE:
        _CACHE["nc"] = _build()
    return _CACHE["nc"]


def kernel(x, y, mask, Wq, bq, Wkv, bkv, Wp, bp):
    _install_ntff_hook()
    from concourse.bass_utils import run_bass_kernel_spmd

    x = np.asarray(x, dtype=np.float32)
    y = np.asarray(y, dtype=np.float32)
    mask = np.asarray(mask)
    Wq = np.asarray(Wq, dtype=np.float32)
    Wkv = np.asarray(Wkv, dtype=np.float32)
    Wp = np.asarray(Wp, dtype=np.float32)
    bq = np.asarray(bq, dtype=np.float32)
    bkv = np.asarray(bkv, dtype=np.float32)
    bp = np.asarray(bp, dtype=np.float32)

    scale = D ** -0.5
    bf16 = ml_dtypes.bfloat16
    xTs = [np.ascontiguousarray(x[b].T).astype(bf16) for b in range(B)]
    yTs = [np.ascontiguousarray(y[b].T).astype(bf16) for b in range(B)]
    m01Ts = [
        np.ascontiguousarray((~mask[b, 0]).T.astype(np.float32)).astype(bf16)
        for b in range(B)
    ]
    wqTs, wkTs, wvTs, wpTs, bqs = [], [], [], [], []
    for hg in range(2):
        rows = slice(hg * CO, hg * CO + CO)
        wqTs.append(np.ascontiguousarray((Wq[rows] * scale).T).astype(bf16))
        wkTs.append(np.ascontiguousarray(Wkv[rows].T).astype(bf16))
        wvTs.append(np.ascontiguousarray(
            Wkv[C + hg * CO: C + hg * CO + CO].T).astype(bf16))
        wpTs.append(np.ascontiguousarray(Wp[:, rows].T).astype(bf16))
        bqs.append(np.ascontiguousarray(bq[rows] * scale))

    in_maps = []
    for c in range(N_CORES):
        b, hg = divmod(c, 2)
        in_maps.append({
            "xT": xTs[b], "yT": yTs[b], "m01T": m01Ts[b],
            "wqT": wqTs[hg], "wkT": wkTs[hg], "wvT": wvTs[hg],
            "wpT": wpTs[hg], "bq": bqs[hg],
        })

    nc = _get_nc()
    trace = os.environ.get("CC_ATTN_TRACE", "") == "1"
    res = run_bass_kernel_spmd(nc, in_maps, core_ids=list(range(N_CORES)),
                               trace=trace)
    _CACHE["last_result"] = res

    # host gather: sum the two head-group partials per batch + exact bias folds
    bias = bkv[C:] @ Wp.T + bp  # v-bias passes through softmax exactly
    out = np.empty((B, NQ, C), dtype=np.float32)
    for b in range(B):
        out[b] = (res.results[2 * b]["part"].astype(np.float32)
                  + res.results[2 * b + 1]["part"].astype(np.float32) + bias)
    return out


# revision 26
# speedup vs baseline: 1.0436x; 1.0016x over previous
"""Cross-attention kernel for 8 Trainium2 NeuronCores (Bass/Tile, SPMD).

Problem: nn_CrossAttention (B=4, NQ=1024, NK=2048, C=1024, H=16, D=64), fp32.

Sharding: (batch x head-group) across the 8 cores. Core c handles batch
b = c//2 and heads h0 = (c%2)*8 .. h0+8 (column-parallel q/k/v projections,
row-parallel output projection). Each core emits a partial output
projection [NQ, C]; the host sums the two partials per batch (+ biases).

Device dataflow is fully "feature-major" (transposed): the host passes
x.T / y.T / W.T so every matmul contraction runs over the SBUF partition
axis with no on-device transposes:

  qT[o,t]  = sum_c wqT[c,o] * xT[c,t]          (o-blocks of 128 = 2 heads)
  kT[o,s]  = sum_c wkT[c,o] * yT[c,s]
  v[s,o]   = sum_c yT[c,s] * wvT[c,o]          (token-major)
  ST[s,t]  = sum_d kT_h[d,s-chunk] * qT_h[d,t]   per head (K=64)
  P[s,t]   = exp(ST) * mask01[s,t]               (ACT exp from PSUM, DVE mul)
  out_aug  = sum_s [ones | v_h]^T P  -> rows 0:64 denominator (x64
             replicated), rows 64:128 numerator
  outF     = out_aug[64:128] * recip(out_aug[0:64])   (custom-DVE recip
             reads PSUM directly; no partition broadcast / row extract)
  partial[t,co] = sum_o outF[o,t-block] * wpT[o,co]

All projections are bf16 (fp8 was tried: softmax score noise transfers
~1:1 into the output - no sqrt(N) washout - and e4m3's ~3% per-operand
quantization lands the end-to-end error at 3.2e-2 > the 2e-2 gate).

Schedule: fully software-pipelined. The attention loop processes the 8
heads sequentially; v-projection and the kT/qT projections for later head
pairs run as PE "filler" inside the chunk loop so the tensor engine stays
busy while the ACT engine drains the exp stream. LA=3 chunk lookahead
covers the exp+mask latency and the early-DMA arrival of v. Iterations
with no filler emit one dummy matmul - without it the PE micro-idles every
chunk in the ACT-bound passes and the HAM clock-gate drops the PE to 1.2
GHz (measured: ~70us of half-clock). Lead-in DMAs are spread across five
engine queues so the first-score critical path (~5MB of y/wk/x/wq/m0) does
not serialize on one queue. PSUM: 2 banks projections + 4 banks scores
(2 in flight) + 2 banks attn-out = 8.

Bias handling (exact): bq is added on-device during the qT eviction;
bk shifts every score of a row equally -> softmax-invariant -> dropped;
bv passes through the softmax average exactly -> host adds bv @ Wp.T;
bp is added on the host.
"""

import os
import sys

if "/opt/trn_rl_repo" not in sys.path:
    sys.path.insert(0, "/opt/trn_rl_repo")

import numpy as np
import ml_dtypes

B, NQ, NK, C, H = 4, 1024, 2048, 1024, 16
D = C // H          # 64
HC = H // 2         # 8 heads per core
CO = HC * D         # 512 output dims per core
N_CORES = 8

_CACHE = {}


def _install_ntff_hook():
    """Register the axon NTFF profile hook (missing antenv.axon_hooks shim).
    Only needed when tracing; harmless otherwise."""
    import types

    if "antenv.axon_hooks" in sys.modules:
        return
    state = {"hook": None}
    mod = types.ModuleType("antenv.axon_hooks")
    mod.set_axon_ntff_profile_hook = lambda h: state.__setitem__("hook", h)
    mod.get_axon_ntff_profile_hook = lambda: state["hook"]
    sys.modules["antenv.axon_hooks"] = mod
    try:
        from trn_agent_boot.trn_boot import _ntff_profile_via_ctypes

        mod.set_axon_ntff_profile_hook(
            _ntff_profile_via_ctypes("/opt/axon/libaxon_pjrt.so")
        )
    except Exception:
        pass


def _build():
    import concourse.mybir as mybir
    import concourse.tile as tile
    from concourse import bacc

    F32 = mybir.dt.float32
    BF16 = mybir.dt.bfloat16
    Exp = mybir.ActivationFunctionType.Exp

    nc = bacc.Bacc("TRN2", target_bir_lowering=False, debug=False,
                   num_devices=N_CORES)

    def din(name, shape, dt=BF16):
        return nc.dram_tensor(name, shape, dt, kind="ExternalInput").ap()

    # all big inputs pre-chunked on the host to [128, n_chunks, W] so each
    # logical tensor loads with ONE DMA trigger (per-chunk triggers pay
    # ~0.6-0.9us each through queue flow control and wreck the lead-in)
    xT = din("xT", [128, 8, NQ])       # x[b].T chunked
    yT = din("yT", [128, 8, NK])       # y[b].T chunked
    m01T = din("m01T", [128, 16, NQ])  # keep=1 / masked=0, T, chunked
    wqT = din("wqT", [128, 8, CO])     # (Wq[rows]*scale).T chunked
    wkT = din("wkT", [128, 8, CO])
    wvT = din("wvT", [128, 8, CO])
    wpT = din("wpT", [128, 4, C])      # Wp[:, rows].T chunked
    bqv = din("bq", [CO], mybir.dt.float32)   # scaled bq slice
    part = nc.dram_tensor("part", [NQ, C], BF16, kind="ExternalOutput").ap()

    LA = 3  # score -> attn-v lookahead (chunks in flight)

    with tile.TileContext(nc) as tc:
        with (
            tc.tile_pool(name="persist", bufs=1) as persist,
            tc.tile_pool(name="work_p", bufs=4) as pp_,
            tc.tile_pool(name="work_d", bufs=1) as pd_,
            tc.tile_pool(name="ps_proj", bufs=2, space="PSUM") as ps_proj,
            tc.tile_pool(name="ps_st", bufs=2, space="PSUM") as ps_st,
            tc.tile_pool(name="ps_out", bufs=1, space="PSUM") as ps_out,
        ):
            # ---- persistent tiles --------------------------------------
            x3 = persist.tile([128, 8, NQ], BF16, tag="x3", name="x3")
            y3 = persist.tile([128, 8, NK], BF16, tag="y3", name="y3")
            wq3 = persist.tile([128, 8, CO], BF16, tag="wq3", name="wq3")
            wk3 = persist.tile([128, 8, CO], BF16, tag="wk3", name="wk3")
            wv3 = persist.tile([128, 8, CO], BF16, tag="wv3", name="wv3")
            wp3 = persist.tile([128, 4, C], BF16, tag="wp3", name="wp3")
            m3 = persist.tile([128, 16, NQ], BF16, tag="m3", name="m3")
            x_sb = [x3[:, c, :] for c in range(8)]
            y_sb = [y3[:, c, :] for c in range(8)]
            wq_sb = [wq3[:, c, :] for c in range(8)]
            wk_sb = [wk3[:, c, :] for c in range(8)]
            wv_sb = [wv3[:, c, :] for c in range(8)]
            wp_sb = [wp3[:, i, :] for i in range(4)]
            m_sb = [m3[:, i, :] for i in range(16)]
            kT_sb = [persist.tile([128, NK], BF16, tag=f"kT{i}", name=f"kT{i}")
                     for i in range(4)]
            qT_sb = [persist.tile([128, NQ], BF16, tag=f"qT{i}", name=f"qT{i}")
                     for i in range(4)]
            # v: 16 interleaved blocks [ones|v_0|ones|v_1|...]: block 2h =
            # ones, block 2h+1 = v_h[s,d]. Stationary for head h is the
            # contiguous pair [:, 2h:2h+2, :] = [ones | v_h] -> out rows
            # 0:64 denominator, 64:128 numerator. (The BIR weights AP must
            # be a single free dim, so the pair must be contiguous; the
            # denominator must land on partition base 0 because the custom-
            # DVE reciprocal requires matching in/out partition bases.)
            v_sb = [persist.tile([128, 16, D], BF16, tag=f"v{i}", name=f"v{i}")
                    for i in range(16)]
            outF_sb = [persist.tile([128, NQ], BF16, tag=f"oF{i}",
                                    name=f"oF{i}") for i in range(4)]
            bq_sb = [persist.tile([128, 1], F32, tag=f"bq{i}", name=f"bq{i}")
                     for i in range(4)]
            po032 = [persist.tile([128, C], BF16, tag=f"po032_{tb}",
                                  name=f"po032_{tb}") for tb in range(2, 8)]
            warm = persist.tile([1, 8], F32, tag="warm", name="warm")
            warm2 = persist.tile([1, 8], F32, tag="warm2", name="warm2")
            # exp bias passed as our own zeroed AP: a float bias would lower
            # to a framework const-AP whose DMA queues behind all the input
            # DMAs - it blocked the first exp (and the whole score pipeline
            # behind the 2-buffer PSUM) until ~35us.
            zb = persist.tile([128, 1], F32, tag="zb", name="zb")

            # dummy exp first: forces the ACT exp-table load (~2.7us) at
            # t~1us. The ones blocks of v go on the scalar queue too - ACT
            # is idle until the first real exp, while the vector queue must
            # stay clear for the first kT/qT evictions.
            nc.vector.memset(warm[:], 0.0)
            nc.vector.memset(zb[:], 0.0)
            nc.scalar.activation(warm2[:], warm[:], Exp, bias=zb[0:1, :])

            # ---- input DMAs, in first-needed order ---------------------
            # The first-score critical path (y s-cols 0:512, wk, x, wq, m0)
            # is spread across five engine queues so it lands in ~1/4 the
            # serial time; everything later streams on the sync queue in
            # consumption order (v units from iter ~2 need wv + leading y
            # columns; kT(0,1..3) needs the rest of y).
            # strict arrival-priority order; transfers are aggregate-
            # bandwidth-bound (~0.33 GB/us/core), so order == arrival time.
            # gpsimd takes the mid-priority items so the sync queue reaches
            # the y tail sooner.
            nc.sync.dma_start(y3[:, :, 0:512], yT[:, :, 0:512])
            nc.sync.dma_start(wk3[:], wkT)
            nc.sync.dma_start(x3[:, :, 0:512], xT[:, :, 0:512])
            nc.sync.dma_start(wq3[:], wqT)
            for ob in range(4):
                nc.sync.dma_start(bq_sb[ob][:],
                                  bqv[ob * 128:(ob + 1) * 128][:, None])
            nc.sync.dma_start(m3[:, 0:3, :], m01T[:, 0:3, :])
            nc.gpsimd.dma_start(x3[:, :, 512:NQ], xT[:, :, 512:NQ])
            nc.gpsimd.dma_start(wv3[:], wvT)
            nc.sync.dma_start(y3[:, :, 512:1024], yT[:, :, 512:1024])
            nc.gpsimd.dma_start(m3[:, 3:8, :], m01T[:, 3:8, :])
            nc.sync.dma_start(y3[:, :, 1024:1536], yT[:, :, 1024:1536])
            nc.sync.dma_start(y3[:, :, 1536:2048], yT[:, :, 1536:2048])
            nc.sync.dma_start(m3[:, 8:16, :], m01T[:, 8:16, :])
            nc.sync.dma_start(wp3[:], wpT)

            # ---- PE unit emitters --------------------------------------
            # each projection unit is split into two "chunks" of 4 matmuls
            # so the filler can be paced finely inside the attention loop
            def _proj_chunks(make_stationary, moving, evict, name):
                box = {}

                def c1():
                    box["t"] = ps_proj.tile([128, 512], F32, tag="pps",
                                            name=name)
                    for cc in range(4):
                        nc.tensor.matmul(
                            box["t"][:], make_stationary(cc), moving(cc),
                            start=(cc == 0), stop=False,
                            skip_group_check=True,
                        )

                def c2():
                    for cc in range(4, 8):
                        nc.tensor.matmul(
                            box["t"][:], make_stationary(cc), moving(cc),
                            start=False, stop=(cc == 7),
                            skip_group_check=True,
                        )
                    evict(box["t"])

                return [c1, c2]

            def qproj_chunks(ob, tc2):
                return _proj_chunks(
                    lambda cc: wq_sb[cc][:, ob * 128:(ob + 1) * 128],
                    lambda cc: x_sb[cc][:, tc2 * 512:(tc2 + 1) * 512],
                    lambda t: nc.vector.tensor_scalar_add(
                        qT_sb[ob][:, tc2 * 512:(tc2 + 1) * 512],
                        t[:], bq_sb[ob][:]),
                    f"qps{ob}_{tc2}",
                )

            def kT_chunks(ob, sc4):
                return _proj_chunks(
                    lambda cc: wk_sb[cc][:, ob * 128:(ob + 1) * 128],
                    lambda cc: y_sb[cc][:, sc4 * 512:(sc4 + 1) * 512],
                    lambda t: nc.vector.tensor_copy(
                        kT_sb[ob][:, sc4 * 512:(sc4 + 1) * 512], t[:]),
                    f"kps{ob}_{sc4}",
                )

            def v_chunks(sc):
                return _proj_chunks(
                    lambda cc: y_sb[cc][:, sc * 128:(sc + 1) * 128],
                    lambda cc: wv_sb[cc][:],
                    lambda t: nc.vector.tensor_copy(v_sb[sc][:, 1:16:2, :],
                                                    t[:]),
                    f"vps{sc}",
                )

            def dummy_unit(tag):
                # keeps the PE issue queue deep in filler-less iterations so
                # the tensor engine holds its high p-state; result never read
                dps = ps_proj.tile([128, 512], F32, tag="pps", name=tag)
                nc.tensor.matmul(
                    dps[:], kT_sb[0][0:64, 0:128], qT_sb[0][0:64, 0:512],
                    start=True, stop=True, skip_group_check=True,
                )

            # out-projection pre-pass for token block tb: oc 0..2 staged to
            # SBUF bf16; legal filler any time after pass (2,1)'s normalize.
            def dpre_chunks(tb):
                def one(co):
                    def c():
                        t = ps_proj.tile([128, 512], F32, tag="pps",
                                         name=f"dpre{tb}_{co}")
                        for oc in range(3):
                            nc.tensor.matmul(
                                t[:],
                                outF_sb[oc][:, tb * 128:(tb + 1) * 128],
                                wp_sb[oc][:, co * 512:(co + 1) * 512],
                                start=(oc == 0), stop=(oc == 2),
                                skip_group_check=True,
                            )
                        nc.vector.tensor_copy(
                            po032[tb - 2][:, co * 512:(co + 1) * 512], t[:])
                    return c
                return [one(0), one(1)]

            # ---- startup: kT(ob0,s0..3-part) + qT(ob0) -----------------
            for c in kT_chunks(0, 0):
                c()
            for tc2 in range(2):
                for c in qproj_chunks(0, tc2):
                    c()
            # v ones blocks: emitted after the startup units so the vector
            # queue runs the first kT/qT evictions before these 16 memsets
            for sc in range(16):
                nc.vector.memset(v_sb[sc][:, 0:16:2, :], 1.0)

            # filler schedule.  Pass (0,0) uses an explicit per-iteration
            # list tuned to deadlines (v unit sc emitted by iter sc+2, one
            # iter before AV(sc) at iter sc+LA; kT(0,sb) before score chunk
            # 4sb) and DMA arrival (nothing in iters 0-1).  Other passes
            # pace a flat list evenly; empty iterations emit a dummy.
            def p00_filler():
                k1, k2, k3 = kT_chunks(0, 1), kT_chunks(0, 2), kT_chunks(0, 3)
                v = [v_chunks(sc) for sc in range(16)]
                return [
                    [],                                       # it0
                    [],                                       # it1
                    [v[0][0], v[0][1]],                       # it2
                    [v[1][0], v[1][1], k1[0]],                # it3
                    [k1[1], v[2][0], v[2][1]],                # it4
                    [v[3][0], v[3][1], v[4][0]],              # it5
                    [v[4][1], v[5][0], v[5][1]],              # it6
                    [v[6][0], v[6][1], k2[0]],                # it7
                    [k2[1], v[7][0], v[7][1]],                # it8
                    [v[8][0], v[8][1], v[9][0]],              # it9
                    [v[9][1], v[10][0], v[10][1]],            # it10
                    [v[11][0], v[11][1], k3[0]],              # it11
                    [k3[1], v[12][0], v[12][1]],              # it12
                    [v[13][0], v[13][1]],                     # it13
                    [v[14][0], v[14][1]],                     # it14
                    [v[15][0], v[15][1]],                     # it15
                ]

            def flat(units):
                return [c for u in units for c in u]

            filler = {
                (0, 0): p00_filler(),
                (0, 1): flat([qproj_chunks(1, 0), kT_chunks(1, 0),
                              qproj_chunks(1, 1), kT_chunks(1, 1),
                              kT_chunks(1, 2), kT_chunks(1, 3)]),
                (1, 0): flat([qproj_chunks(2, 0), kT_chunks(2, 0),
                              kT_chunks(2, 1)]),
                (1, 1): flat([qproj_chunks(2, 1), kT_chunks(2, 2),
                              kT_chunks(2, 3)]),
                (2, 0): flat([qproj_chunks(3, 0), kT_chunks(3, 0),
                              kT_chunks(3, 1)]),
                (2, 1): flat([qproj_chunks(3, 1), kT_chunks(3, 2),
                              kT_chunks(3, 3)]),
                (3, 0): flat([dpre_chunks(2), dpre_chunks(3), dpre_chunks(4)]),
                (3, 1): flat([dpre_chunks(5), dpre_chunks(6), dpre_chunks(7)]),
            }

            # ---- attention: heads sequential, pipelined chunks ---------
            for hp in range(4):
                for h2 in range(2):
                    h = 2 * hp + h2
                    p0 = h2 * 64
                    fl = filler[(hp, h2)]
                    explicit = bool(fl) and isinstance(fl[0], list)
                    n_chunks = 0 if explicit else len(fl)
                    popped = 0
                    outps = ps_out.tile([128, NQ], F32, tag="outps",
                                        name=f"outps{hp}_{h2}")
                    pts = {}
                    for it in range(16 + LA):
                        if it >= LA:
                            # attn-v first: its inputs (pt, v) are LA chunks
                            # old, so the PE never stalls entering the iter
                            sc = it - LA
                            pt = pts.pop(sc)
                            vst = v_sb[sc][:, 2 * h:2 * h + 2, :]
                            for tc2 in range(2):
                                nc.tensor.matmul(
                                    outps[:, tc2 * 512:(tc2 + 1) * 512],
                                    vst,
                                    pt[:, tc2 * 512:(tc2 + 1) * 512],
                                    start=(sc == 0), stop=(sc == 15),
                                    skip_group_check=True,
                                )
                        if it < 16:
                            sc = it
                            # PE filler ahead of this chunk's score matmuls
                            did = 0
                            if explicit:
                                for cch in fl[it]:
                                    cch()
                                    did += 1
                            else:
                                want = -(-n_chunks * (it + 1) // 16)  # ceil
                                while popped < want:
                                    fl[popped]()
                                    popped += 1
                                    did += 1
                            if did == 0 and it >= 2 and it % 2 == 0:
                                dummy_unit(f"dmy{hp}_{h2}_{it}")
                            stp = ps_st.tile([128, NQ], F32, tag="stp",
                                             name=f"stp{hp}_{h2}_{sc}")
                            for tc2 in range(2):
                                nc.tensor.matmul(
                                    stp[:, tc2 * 512:(tc2 + 1) * 512],
                                    kT_sb[hp][p0:p0 + 64,
                                              sc * 128:(sc + 1) * 128],
                                    qT_sb[hp][p0:p0 + 64,
                                              tc2 * 512:(tc2 + 1) * 512],
                                    start=True, stop=True,
                                    skip_group_check=True,
                                )
                            pt = pp_.tile([128, NQ], BF16, tag="pt")
                            nc.scalar.activation(pt[:], stp[:], Exp,
                                                 bias=zb[:])
                            nc.vector.tensor_mul(pt[:], pt[:], m_sb[sc][:])
                            pts[sc] = pt
                    # normalize: denominator rows 0:64 (x64 replicated),
                    # numerator rows 64:128; custom-DVE recip reads PSUM
                    rcp = pd_.tile([64, NQ], F32, tag="rcp")
                    nc.vector.reciprocal_approx_fast(rcp[:],
                                                     outps[0:64, :])
                    nc.vector.tensor_mul(
                        outF_sb[hp][p0:p0 + 64, :], outps[64:128, :], rcp[:])

            # ---- output projection -------------------------------------
            # tb0/tb1 pre-accumulate oc=0..2 while the final normalize
            # (which produces outF[3] rows 64:128) is still in flight, so
            # the PE keeps running through the attention->projection seam.
            with tc.tile_pool(name="proj", bufs=2) as prj:
                def d_mms(tb, pps, oc_lo, oc_hi):
                    for oc in range(oc_lo, oc_hi):
                        for co in range(2):
                            nc.tensor.matmul(
                                pps[:, co * 512:(co + 1) * 512],
                                outF_sb[oc][:, tb * 128:(tb + 1) * 128],
                                wp_sb[oc][:, co * 512:(co + 1) * 512],
                                start=(oc == 0), stop=(oc == 3),
                                skip_group_check=True,
                            )

                dma_eng = [nc.sync, nc.scalar]

                pps_of = {}
                for tb in range(2):
                    pps_of[tb] = ps_st.tile([128, NQ], F32, tag="stp",
                                            name=f"pps{tb}")
                    d_mms(tb, pps_of[tb], 0, 3)
                # tail: oc3 matmuls burst on the PE; tb0/tb1 evict via ACT
                # copies, tb2-7 add in place into their staged po032 tiles
                # on DVE; the 8 output DMAs alternate over two queues
                # (gpsimd descriptor generation is too slow for the tail)
                for tb in range(2):
                    po = prj.tile([128, C], BF16, tag="po")
                    d_mms(tb, pps_of[tb], 3, 4)
                    nc.scalar.copy(po[:], pps_of[tb][:])
                    dma_eng[tb % 2].dma_start(
                        part[tb * 128:(tb + 1) * 128, :], po[:])
                for tb in range(2, 8):
                    for co in range(2):
                        t = ps_proj.tile([128, 512], F32, tag="pps",
                                         name=f"dfin{tb}_{co}")
                        nc.tensor.matmul(
                            t[:],
                            outF_sb[3][:, tb * 128:(tb + 1) * 128],
                            wp_sb[3][:, co * 512:(co + 1) * 512],
                            start=True, stop=True,
                            skip_group_check=True,
                        )
                        nc.vector.tensor_add(
                            po032[tb - 2][:, co * 512:(co + 1) * 512], t[:],
                            po032[tb - 2][:, co * 512:(co + 1) * 512])
                    dummy_unit(f"dmyfin{tb}")
                    dma_eng[tb % 2].dma_start(
                        part[tb * 128:(tb + 1) * 128, :], po032[tb - 2][:])

    nc.compile()
    return nc


def _get_nc():
    if "nc" not in _CACHE:
        _CACHE["nc"] = _build()
    return _CACHE["nc"]


def kernel(x, y, mask, Wq, bq, Wkv, bkv, Wp, bp):
    _install_ntff_hook()
    from concourse.bass_utils import run_bass_kernel_spmd

    x = np.asarray(x, dtype=np.float32)
    y = np.asarray(y, dtype=np.float32)
    mask = np.asarray(mask)
    Wq = np.asarray(Wq, dtype=np.float32)
    Wkv = np.asarray(Wkv, dtype=np.float32)
    Wp = np.asarray(Wp, dtype=np.float32)
    bq = np.asarray(bq, dtype=np.float32)
    bkv = np.asarray(bkv, dtype=np.float32)
    bp = np.asarray(bp, dtype=np.float32)

    scale = D ** -0.5
    bf16 = ml_dtypes.bfloat16

    def chunk(aT):
        # [R, W] -> [128, R//128, W]: row r = chunk*128+p -> [p, chunk, :]
        R, W = aT.shape
        return np.ascontiguousarray(
            aT.reshape(R // 128, 128, W).transpose(1, 0, 2))

    xTs = [chunk(x[b].T.astype(bf16)) for b in range(B)]
    yTs = [chunk(y[b].T.astype(bf16)) for b in range(B)]
    m01Ts = [
        chunk((~mask[b, 0]).T.astype(np.float32).astype(bf16))
        for b in range(B)
    ]
    wqTs, wkTs, wvTs, wpTs, bqs = [], [], [], [], []
    for hg in range(2):
        rows = slice(hg * CO, hg * CO + CO)
        wqTs.append(chunk((Wq[rows] * scale).T.astype(bf16)))
        wkTs.append(chunk(Wkv[rows].T.astype(bf16)))
        wvTs.append(chunk(
            Wkv[C + hg * CO: C + hg * CO + CO].T.astype(bf16)))
        wpTs.append(chunk(Wp[:, rows].T.astype(bf16)))
        bqs.append(np.ascontiguousarray(bq[rows] * scale))

    in_maps = []
    for c in range(N_CORES):
        b, hg = divmod(c, 2)
        in_maps.append({
            "xT": xTs[b], "yT": yTs[b], "m01T": m01Ts[b],
            "wqT": wqTs[hg], "wkT": wkTs[hg], "wvT": wvTs[hg],
            "wpT": wpTs[hg], "bq": bqs[hg],
        })

    nc = _get_nc()
    trace = os.environ.get("CC_ATTN_TRACE", "") == "1"
    res = run_bass_kernel_spmd(nc, in_maps, core_ids=list(range(N_CORES)),
                               trace=trace)
    _CACHE["last_result"] = res

    # host gather: sum the two head-group partials per batch + exact bias folds
    bias = bkv[C:] @ Wp.T + bp  # v-bias passes through softmax exactly
    out = np.empty((B, NQ, C), dtype=np.float32)
    for b in range(B):
        out[b] = (res.results[2 * b]["part"].astype(np.float32)
                  + res.results[2 * b + 1]["part"].astype(np.float32) + bias)
    return out
